# revision 20
# baseline (speedup 1.0000x reference)
"""Trainium2 Bass kernel for Qwen-style GQA attention block (B=2,S=2048,H=16,KV=8,D=128).

Sharding (8 cores): batch(2) x si-stripes(2) x head-half(2).
  core c: b=c>>2, sh=(c>>1)&1, hh=c&1
  - each core projects Q/K/V for ITS stripe rows only (1024 tokens); K/V results
    (roped, transposed, normalized) are exchanged between the two stripe cores
    via a small AllGather so both see full-S K/V.
  - attention j-outer with wide score tiles (stationary K-block reuse, wide exp).
  - pair AllGather of ctx^T split in two head-groups, column-split o_proj in two
    passes so the second collective hides under the first o_proj pass.
All matmuls bf16 with fp32 PSUM accumulation. Softmax without max-subtraction
(scores are O(1) after QK RMSNorm); denominator via an appended ones-column on V.
"""
import sys

sys.path.insert(0, '/opt/trn_rl_repo')

import numpy as np

import concourse.bass as bass
import concourse.tile as tile
from concourse import mybir
from concourse.vector_clock import ScopedClock, VectorClock

B, S, HID = 2, 2048, 2048
H, KV, D = 16, 8, 128
EPS = 1e-6
SCALE = D ** -0.5
NBLK = S // 128  # 16
# causally balanced si-block stripes: sum(i+1) = 68 for both
MYBLKS = [[0, 2, 4, 6, 9, 11, 13, 15], [1, 3, 5, 7, 8, 10, 12, 14]]
BOUNDS = [max(MYBLKS[0][bi], MYBLKS[1][bi]) for bi in range(8)]  # [1,3,..,15]
# o_proj ctx row order after the two pair-AllGathers (global head ids)
OHEAD_ORDER = [0, 1, 8, 9, 2, 3, 10, 11, 4, 5, 12, 13, 6, 7, 14, 15]

F32 = mybir.dt.float32
BF16 = mybir.dt.bfloat16
AF = mybir.ActivationFunctionType
MUL = mybir.AluOpType.mult
ADD = mybir.AluOpType.add


# ---------------------------------------------------------------------------
# Workarounds: this walrus supports only ONE sync-wait per instruction.
def _patched_drain_and_barrier(self, tick_clock, wait_clock):
    gc = tick_clock.global_clock
    vec = list(gc)
    nz = [i for i, v in enumerate(vec) if v > 0] or [0]
    for i in nz:
        cvec = [vec[j] if j == i else 0 for j in range(len(vec))]
        inst = self.nc.sync.drain()
        wait_clock.add_sem_waits(inst.ins, ScopedClock({None: VectorClock(cvec)}))
    self.nc.all_engine_barrier()
    assert self.sems is not None
    popped = self.nc._tile_sem_poison_stack.pop()
    assert popped is self._sem_poison
    self.nc.clear_and_free_semaphores(list(self.sems.allocated().values()))
    self.nc.all_engine_barrier()


tile.TileContext._drain_and_barrier = _patched_drain_and_barrier


def split_multi_waits(nc):
    for fn in nc.m.functions:
        for blk in fn.blocks:
            insts = list(blk.instructions)
            out = []
            changed = False
            for inst in insts:
                si = inst.sync_info
                if si is not None and len(si.on_wait) > 1:
                    waits = list(si.on_wait)
                    for k, w in enumerate(waits[:-1]):
                        out.append(mybir.InstNoOp(
                            name=f"{inst.name}.w{k}", engine=inst.engine,
                            sync_info=mybir.SyncInfo(on_wait=[w], on_update=[]),
                            text_hint="waitsplit"))
                    si.on_wait = [waits[-1]]
                    changed = True
                out.append(inst)
            if changed:
                blk.instructions[:] = out


# ---------------------------------------------------------------------------
def build_kernel():
    nc = bass.Bass(trn_type='TRN2')
    # hidden^T for THIS core's stripe rows (local bi-block order)
    hT = nc.dram_tensor('hT', [HID, 1024], F32, kind='ExternalInput')
    qwT = nc.dram_tensor('qwT', [HID, 1024], F32, kind='ExternalInput')
    kwT = nc.dram_tensor('kwT', [HID, 512], F32, kind='ExternalInput')
    vwT = nc.dram_tensor('vwT', [HID, 512], F32, kind='ExternalInput')
    owT = nc.dram_tensor('owT', [2048, 1024], F32, kind='ExternalInput')
    # host-fused rope tables (cos/sin x norm-weight halves), [1024, 4, 64]
    qtab = nc.dram_tensor('qtab', [1024, 4, 64], F32, kind='ExternalInput')
    ktab = nc.dram_tensor('ktab', [1024, 4, 64], F32, kind='ExternalInput')
    iden = nc.dram_tensor('iden', [128, 128], F32, kind='ExternalInput')
    # per-core diagonal masks dm[bi, t] for j in {BOUNDS[bi]-1, BOUNDS[bi]}
    dm = nc.dram_tensor('dm', [8, 2, 128, 128], F32, kind='ExternalInput')
    out_e = nc.dram_tensor('out', [1024, 1024], F32, kind='ExternalOutput')

    from contextlib import ExitStack
    with ExitStack() as ctx:
        tc = ctx.enter_context(tile.TileContext(nc))
        pool = lambda name, bufs, **kw: ctx.enter_context(
            tc.tile_pool(name=name, bufs=bufs, **kw))
        p_c = pool('const', 1)
        p_wv = pool('wv', 4)      # wv tiles, later QT tiles
        p_wk = pool('wk', 4)      # wk tiles, later ctm tiles
        p_wq = pool('wq', 4)      # wq tiles, later wo tiles
        p_h = pool('ht', 4)       # hT tiles, later ctf tiles
        p_kv = pool('kvstage', 1)  # KTh/VAh staging + KT_all/VA_all
        p_w = pool('work', 2)     # rope scratch etc
        p_s = pool('small', 4)
        p_ex = pool('expb', 4)
        p_ob = pool('outb', 2)
        ps_m = pool('psM', 3, space='PSUM')   # proj psums + scores + o_proj
        ps_c = pool('psC', 1, space='PSUM')   # ctx accumulators (3 tags)
        ps_t = pool('psT', 2, space='PSUM')   # transposes
        p_d = pool('dram', 1, space='DRAM')

        # ---- DMA prologue (gpsimd queue): wv/hT first so V proj starts early
        wv_s = [p_wv.tile([128, 4, 512], BF16, tag='wv', name='wv') for _ in range(4)]
        wk_s = [p_wk.tile([128, 4, 512], BF16, tag='wk', name='wk') for _ in range(4)]
        wq_s = [p_wq.tile([128, 4, 1024], BF16, tag='wq', name='wq') for _ in range(4)]
        ht_t = [p_h.tile([128, 4, 1024], BF16, tag='ht', name='ht') for _ in range(4)]
        for g in range(4):
            r = bass.ts(g, 512)
            nc.gpsimd.dma_start(wv_s[g][:], vwT[r, :].rearrange('(n p) c -> p n c', p=128))
            nc.gpsimd.dma_start(ht_t[g][:], hT[r, :].rearrange('(n p) c -> p n c', p=128))
        for g in range(4):
            nc.gpsimd.dma_start(wk_s[g][:], kwT[bass.ts(g, 512), :].rearrange('(n p) c -> p n c', p=128))
        iden_s = p_c.tile([128, 128], BF16)
        nc.gpsimd.dma_start(iden_s[:], iden[:])
        ktab_s = p_c.tile([128, 8, 4, 64], BF16)
        nc.gpsimd.dma_start(ktab_s[:], ktab.rearrange('(n p) t d -> p n t d', p=128))
        qtab_s = p_c.tile([128, 8, 4, 64], BF16)
        nc.gpsimd.dma_start(qtab_s[:], qtab.rearrange('(n p) t d -> p n t d', p=128))
        for g in range(4):
            nc.gpsimd.dma_start(wq_s[g][:], qwT[bass.ts(g, 512), :].rearrange('(n p) c -> p n c', p=128))
        dm_s = p_c.tile([128, 8, 2, 128], BF16)
        nc.gpsimd.dma_start(dm_s[:], dm.rearrange('n t p d -> p n t d'))

        # persistent K/V stores (full S, post-exchange) + local staging
        KTh = p_kv.tile([128, 4, 1024], BF16, tag='kth', name='KTh')
        VAh = p_kv.tile([128, 4, 8, 132], BF16, tag='vah', name='VAh')
        KT = p_kv.tile([128, 4, 2, 1024], BF16, tag='kt', name='KT')
        VA = p_kv.tile([128, 4, 16, 132], BF16, tag='va', name='VA')
        nc.gpsimd.memset(VAh[:, :, :, 128:132], 1.0)

        # ---- V projection: ch-major waves so PE starts on the first DMA chunk
        wave_tags = ['ps', 'ps', 'ps', 'c0', 'c1', 'c2']
        for wave, sbs in ((0, range(0, 6)), (1, range(6, 8))):
            psVs = {}
            for idx, sb in enumerate(sbs):
                tag = wave_tags[idx] if wave == 0 else 'ps'
                pool_ = ps_m if tag == 'ps' else ps_c
                psVs[sb] = pool_.tile([128, 512], F32, tag=tag, name=f'psV{sb}')
            for g in range(4):
                for i in range(4):
                    ch = g * 4 + i
                    for sb in sbs:
                        nc.tensor.matmul(psVs[sb][:], ht_t[g][:, i, bass.ts(sb, 128)],
                                         wv_s[g][:, i, :],
                                         start=(ch == 0), stop=(ch == 15))
            for sb in sbs:
                nc.scalar.copy(VAh[:, :, sb, 0:128],
                               psVs[sb][:].rearrange('p (k d) -> p k d', k=4))

        # ---- K projection + RMSNorm(*SCALE) + rope + transpose ----
        for sb in range(8):
            psK = ps_m.tile([128, 512], F32, tag='ps', name='psK')
            for ch in range(16):
                nc.tensor.matmul(psK[:], ht_t[ch // 4][:, ch % 4, bass.ts(sb, 128)],
                                 wk_s[ch // 4][:, ch % 4, :],
                                 start=(ch == 0), stop=(ch == 15))
            kraw = p_w.tile([128, 4, 128], BF16, tag='raw', name='kraw')
            nc.scalar.copy(kraw[:], psK[:].rearrange('p (k d) -> p k d', k=4))
            sqd = p_w.tile([128, 4, 128], BF16, tag='sqd', name='sqd')
            nc.vector.tensor_mul(sqd[:], kraw[:], kraw[:])
            ms = p_s.tile([128, 4], F32, tag='ms', name='ms')
            nc.vector.tensor_reduce(ms[:], sqd[:], mybir.AxisListType.X, ADD)
            nc.vector.tensor_scalar_add(ms[:], ms[:], float(EPS * D))
            std = p_s.tile([128, 4], F32, tag='std', name='std')
            nc.scalar.activation(std[:], ms[:], AF.Sqrt, scale=1.0 / D, bias=0.0)
            rstd = p_s.tile([128, 4], F32, tag='rstd', name='rstd')
            nc.vector.reciprocal(rstd[:], std[:])
            rstdS = p_s.tile([128, 4], F32, tag='rstds', name='rstdS')
            nc.vector.tensor_scalar_mul(rstdS[:], rstd[:], SCALE)
            kcs = p_w.tile([128, 4, 128], BF16, tag='kcs', name='kcs')
            for kvh in range(4):
                nc.vector.tensor_scalar_mul(kcs[:, kvh, :], kraw[:, kvh, :],
                                            rstdS[:, kvh:kvh + 1])
            # rope on gpsimd (keeps DVE free); tables already fold k_norm_w
            lo, hi = kcs[:, :, 0:64], kcs[:, :, 64:128]
            tA = ktab_s[:, sb, :, :][:, 0:1, :]
            tB = ktab_s[:, sb, :, :][:, 1:2, :]
            tC = ktab_s[:, sb, :, :][:, 2:3, :]
            tD = ktab_s[:, sb, :, :][:, 3:4, :]
            t_ = p_w.tile([128, 4, 4, 64], BF16, tag='t4', name='t4')
            kro = p_w.tile([128, 4, 128], BF16, tag='kro', name='kro')
            mul_b(nc.gpsimd, t_[:, 0], lo, tA)
            mul_b(nc.gpsimd, t_[:, 1], hi, tB)
            nc.gpsimd.tensor_sub(kro[:, :, 0:64], t_[:, 0], t_[:, 1])
            mul_b(nc.gpsimd, t_[:, 2], hi, tC)
            mul_b(nc.gpsimd, t_[:, 3], lo, tD)
            nc.gpsimd.tensor_add(kro[:, :, 64:128], t_[:, 2], t_[:, 3])
            psTk = ps_t.tile([128, 4, 128], BF16, tag='pst', name='psTk')
            for kvh in range(4):
                nc.tensor.transpose(psTk[:, kvh, :], kro[:, kvh, :], iden_s[:])
            nc.scalar.copy(KTh[:, :, bass.ts(sb, 128)], psTk[:])

        # ---- exchange K/V halves between the stripe pair (hidden under Q) ----
        cckv_in = p_d.tile([128, 8320], BF16, tag='cckvi', name='cckv_in')
        cckv_out = p_d.tile([256, 8320], BF16, tag='cckvo', name='cckv_out')
        nc.sync.dma_start(cckv_in[:, 0:4096], KTh[:])
        nc.sync.dma_start(cckv_in[:, 4096:8320], VAh[:])
        nc.gpsimd.collective_compute(
            'AllGather', mybir.AluOpType.bypass,
            replica_groups=[[0, 1], [2, 3], [4, 5], [6, 7]],
            ins=[cckv_in.opt()], outs=[cckv_out.opt()])
        # read back both stripes, kept in stripe-local order:
        # global block j lives at (rank r_j, slot j//2), r_j = (j%2) ^ (j>=8)
        for r in range(2):
            src = cckv_out[bass.ts(r, 128), :]
            nc.sync.dma_start(KT[:, :, r, :],
                              src[:, 0:4096].rearrange('p (k c) -> p k c', k=4))
            nc.sync.dma_start(VA[:, :, r * 8:(r + 1) * 8, :].rearrange('p k s w -> p k (s w)'),
                              src[:, 4096:8320].rearrange('p (k x) -> p k x', k=4))

        # ---- Q projection (PE busy while exchange completes) ----
        QT = [p_wv.tile([128, 2, 1024], BF16, tag='wv', name='QT') for _ in range(4)]
        for bi in range(8):
            for qg in range(2):
                psQ = ps_m.tile([128, 512], F32, tag='ps', name='psQ')
                for ch in range(16):
                    nc.tensor.matmul(psQ[:], ht_t[ch // 4][:, ch % 4, bass.ts(bi, 128)],
                                     wq_s[ch // 4][:, ch % 4, bass.ts(qg, 512)],
                                     start=(ch == 0), stop=(ch == 15))
                qraw = p_w.tile([128, 4, 128], BF16, tag='raw', name='qraw')
                nc.scalar.copy(qraw[:], psQ[:].rearrange('p (k d) -> p k d', k=4))
                sqd = p_w.tile([128, 4, 128], BF16, tag='sqd', name='sqd')
                nc.vector.tensor_mul(sqd[:], qraw[:], qraw[:])
                ms = p_s.tile([128, 4], F32, tag='ms', name='ms')
                nc.vector.tensor_reduce(ms[:], sqd[:], mybir.AxisListType.X, ADD)
                nc.vector.tensor_scalar_add(ms[:], ms[:], float(EPS * D))
                std = p_s.tile([128, 4], F32, tag='std', name='std')
                nc.scalar.activation(std[:], ms[:], AF.Sqrt, scale=1.0 / D, bias=0.0)
                rstd = p_s.tile([128, 4], F32, tag='rstd', name='rstd')
                nc.vector.reciprocal(rstd[:], std[:])
                qcs = p_w.tile([128, 4, 128], BF16, tag='kcs', name='qcs')
                for hq in range(4):
                    nc.vector.tensor_scalar_mul(qcs[:, hq, :], qraw[:, hq, :],
                                                rstd[:, hq:hq + 1])
                lo, hi = qcs[:, :, 0:64], qcs[:, :, 64:128]
                tA = qtab_s[:, bi, :, :][:, 0:1, :]
                tB = qtab_s[:, bi, :, :][:, 1:2, :]
                tC = qtab_s[:, bi, :, :][:, 2:3, :]
                tD = qtab_s[:, bi, :, :][:, 3:4, :]
                t_ = p_w.tile([128, 4, 4, 64], BF16, tag='t4', name='t4')
                qro = p_w.tile([128, 4, 128], BF16, tag='kro', name='qro')
                eng = nc.vector if qg == 0 else nc.gpsimd
                mul_b(eng, t_[:, 0], lo, tA)
                mul_b(eng, t_[:, 1], hi, tB)
                eng.tensor_sub(qro[:, :, 0:64], t_[:, 0], t_[:, 1])
                mul_b(eng, t_[:, 2], hi, tC)
                mul_b(eng, t_[:, 3], lo, tD)
                eng.tensor_add(qro[:, :, 64:128], t_[:, 2], t_[:, 3])
                psTq = ps_t.tile([128, 4, 128], BF16, tag='pst', name='psTq')
                for hq in range(4):
                    nc.tensor.transpose(psTq[:, hq, :], qro[:, hq, :], iden_s[:])
                nc.scalar.copy(QT[qg * 2][:, :, bass.ts(bi, 128)], psTq[:, 0:2, :])
                nc.scalar.copy(QT[qg * 2 + 1][:, :, bass.ts(bi, 128)], psTq[:, 2:4, :])

        # wo loads (reuse wq slots; runs during attention)
        wo_s = [p_wq.tile([128, 4, 1024], BF16, tag='wq', name='wo') for _ in range(4)]
        for g in range(4):
            nc.gpsimd.dma_start(wo_s[g][:], owT[bass.ts(g, 512), :].rearrange('(n p) c -> p n c', p=128))

        # ---- attention: j-outer, wide score tiles ----
        ctm = [p_wk.tile([128, 2, 1024], BF16, tag='wk', name='ctm') for _ in range(4)]
        # psC slot layout: tag -> (first bi, n slots)
        CGRP = [(0, 3), (3, 3), (6, 2)]

        def cslot(pc, bi):
            g = 0 if bi < 3 else (1 if bi < 6 else 2)
            s = bi - CGRP[g][0]
            return pc[g][:, s * 132:s * 132 + 129]

        ctfs = []
        for h in range(8):
            kvh = h // 2
            qt_ap = QT[h // 2][:, h % 2, :]
            pc = [ps_c.tile([128, CGRP[g][1] * 132], F32, tag=f'c{g}', name=f'pc{g}')
                  for g in range(3)]
            exs = {}
            psTc = None
            for it in range(17):
                # QK for j=it (chunks), exp, mask
                if it < 16:
                    j = it
                    b0 = j // 2
                    if b0 < 4:
                        chunks = [(b0 * 128, 512), (512, 1024)]
                    else:
                        chunks = [(b0 * 128, 1024)]
                    rj = (j % 2) ^ (1 if j >= 8 else 0)
                    kt_ap = KT[:, kvh, rj, bass.ts(j // 2, 128)]
                    cur = []
                    for (c0, c1) in chunks:
                        w = c1 - c0
                        psS = ps_m.tile([128, 512], F32, tag='ps', name='psS')
                        nc.tensor.matmul(psS[:, 0:w], kt_ap, qt_ap[:, c0:c1],
                                         start=True, stop=True)
                        ex = p_ex.tile([128, 512], BF16, tag='ex', name='ex')
                        nc.scalar.activation(ex[:, 0:w], psS[:, 0:w], AF.Exp)
                        cur.append((c0, c1, ex))
                    # diagonal/overhang mask: block bi=b0, t = j parity
                    nc.vector.tensor_mul(cur[0][2][:, 0:128], cur[0][2][:, 0:128],
                                         dm_s[:, b0, j % 2, :])
                    exs[j] = cur
                # PV for j=it-1 (+finalizations)
                if it > 0:
                    j = it - 1
                    b0 = j // 2
                    rj = (j % 2) ^ (1 if j >= 8 else 0)
                    va_ap = VA[:, kvh, rj * 8 + j // 2, 0:129]
                    for bi in range(b0, 8):
                        (c0, c1, ex) = exs[j][0] if bi * 128 < exs[j][0][1] else exs[j][1]
                        exsub = ex[:, bi * 128 - c0:bi * 128 - c0 + 128]
                        # start=True clears has_written for the WHOLE bank: issue it
                        # only on the bank's first matmul; other slots first-write
                        # via the overwrite-where-unwritten path.
                        nc.tensor.matmul(cslot(pc, bi), exsub, va_ap,
                                         start=(j == 0 and bi in (0, 3, 6)),
                                         stop=(j == BOUNDS[bi]),
                                         skip_group_check=True)
                    del exs[j]
                    # finalize bi whose last block was j
                    if it % 2 == 0:
                        bi = (it - 2) // 2
                        sl = cslot(pc, bi)
                        rd = p_s.tile([128, 1], F32, tag='rd', name='rd')
                        nc.vector.reciprocal(rd[:], sl[:, 128:129])
                        cn = p_w.tile([128, 128], BF16, tag='cn', name='cn')
                        nc.vector.tensor_scalar_mul(cn[:], sl[:, 0:128], rd[:])
                        if psTc is None:
                            psTc = ps_t.tile([128, 4, 128], BF16, tag='pst', name='psTc')
                        nc.tensor.transpose(psTc[:, bi % 2, :], cn[:], iden_s[:])
                        if bi % 2 == 1:
                            nc.vector.tensor_scalar_add(
                                ctm[h // 2][:, h % 2, bass.ts(bi // 2, 256)],
                                psTc[:, 0:2, :], 0.0)
                            psTc = None
            # ctx pair-AllGather in 4 chunks (after h1/h3/h5/h7): the slower
            # cross-device exchange hides under remaining attention + o_proj
            if h % 2 == 1:
                p = h // 2
                cci = p_d.tile([256, 1024], BF16, tag=f'cci{p}', name=f'cci{p}')
                cco = p_d.tile([512, 1024], BF16, tag=f'cco{p}', name=f'cco{p}')
                nc.sync.dma_start(cci[:].rearrange('(c p) s -> p c s', c=2), ctm[p][:])
                nc.gpsimd.collective_compute(
                    'AllGather', mybir.AluOpType.bypass,
                    replica_groups=[[0, 2], [1, 3], [4, 6], [5, 7]],
                    ins=[cci.opt()], outs=[cco.opt()])
                pool_f, tag_f = (p_h, 'ht') if p < 2 else (p_wv, 'wv')
                cf = [pool_f.tile([128, 2, 1024], BF16, tag=tag_f, name=f'ctf{p}')
                      for _ in range(2)]
                for i in range(2):
                    nc.sync.dma_start(
                        cf[i][:],
                        cco[bass.ts(i, 256), :].rearrange('(c p) s -> p c s', c=2))
                ctfs.append(cf)

        # ---- o_proj: 4 passes (one per ctx chunk), accumulate in SBUF ----
        obuf = p_kv.tile([128, 8, 1024], BF16, tag='obuf', name='obuf')
        for p in range(4):
            cf = ctfs[p]
            for bi in range(8):
                for nt in range(2):
                    psO = ps_m.tile([128, 512], F32, tag='ps', name='psO')
                    for c in range(4):
                        nc.tensor.matmul(psO[:], cf[c // 2][:, c % 2, bass.ts(bi, 128)],
                                         wo_s[p][:, c, bass.ts(nt, 512)],
                                         start=(c == 0), stop=(c == 3))
                    dst = obuf[:, bi, bass.ts(nt, 512)]
                    if p == 0:
                        nc.vector.tensor_scalar_add(dst, psO[:], 0.0)
                    elif p < 3:
                        nc.vector.tensor_add(dst, psO[:], dst)
                    else:
                        ob = p_ob.tile([128, 512], F32, tag='ob', name='ob')
                        nc.vector.tensor_add(ob[:], psO[:], dst)
                        nc.sync.dma_start(out_e[bass.ts(bi, 128), bass.ts(nt, 512)], ob[:])

    split_multi_waits(nc)
    return nc


def mul_b(eng, out, a, b):
    """tensor_tensor multiply with free-dim broadcast of b over dim 1."""
    a2, b2 = bass.broadcast_tensor_aps(a, b)
    eng.tensor_mul(out, a2, b2)


# ---------------------------------------------------------------------------
_NC_CACHE = None
_LAST_IN_MAPS = None


def _get_nc():
    global _NC_CACHE
    if _NC_CACHE is None:
        _NC_CACHE = build_kernel()
    return _NC_CACHE


def kernel(hidden_states, cos, sin, q_w, k_w, v_w, o_w, q_norm_w, k_norm_w):
    from concourse.bass_utils import run_bass_kernel_spmd

    hidden_states = np.asarray(hidden_states, np.float32)
    cos = np.asarray(cos, np.float32)
    sin = np.asarray(sin, np.float32)
    q_w = np.asarray(q_w, np.float32)
    k_w = np.asarray(k_w, np.float32)
    v_w = np.asarray(v_w, np.float32)
    o_w = np.asarray(o_w, np.float32)
    q_norm_w = np.asarray(q_norm_w, np.float32)
    k_norm_w = np.asarray(k_norm_w, np.float32)

    tri_np = np.triu(np.ones((128, 128), np.float32))  # [sj,si]: valid sj<=si
    iden_np = np.eye(128, dtype=np.float32)
    operm = np.concatenate([np.arange(h * 128, (h + 1) * 128) for h in OHEAD_ORDER])

    def rope_tabs(c, s_, w):
        # tables [rows, 4, 64]: A=c_lo*w_lo, B=s_lo*w_hi, C=c_lo*w_hi, D=s_lo*w_lo
        cl, sl = c[:, 0:64], s_[:, 0:64]
        wl, wh = w[0:64], w[64:128]
        return np.stack([cl * wl, sl * wh, cl * wh, sl * wl], axis=1).astype(np.float32)

    in_maps = []
    for c in range(8):
        b, hh, sh = c >> 2, (c >> 1) & 1, c & 1
        blks = MYBLKS[sh]
        rows = np.concatenate([np.arange(g * 128, (g + 1) * 128) for g in blks])
        hT = np.ascontiguousarray(hidden_states[b][rows].T)
        qwT = np.ascontiguousarray(q_w[hh * 1024:(hh + 1) * 1024].T)
        kwT = np.ascontiguousarray(k_w[hh * 512:(hh + 1) * 512].T)
        vwT = np.ascontiguousarray(v_w[hh * 512:(hh + 1) * 512].T)
        owT = np.ascontiguousarray(o_w[hh * 1024:(hh + 1) * 1024].T[operm])
        qtab = rope_tabs(cos[b][rows], sin[b][rows], q_norm_w)
        ktab = rope_tabs(cos[b][rows], sin[b][rows], k_norm_w)
        # diagonal masks dm[bi, t]: t=0 -> sj block BOUNDS[bi]-1, t=1 -> BOUNDS[bi]
        dm = np.zeros((8, 2, 128, 128), np.float32)
        for bi in range(8):
            g, gb = blks[bi], BOUNDS[bi]
            for t, j in enumerate((gb - 1, gb)):
                if j < g:
                    dm[bi, t] = 1.0
                elif j == g:
                    dm[bi, t] = tri_np
                # j > g: stays 0 (block fully masked)
        in_maps.append(dict(
            hT=hT, qwT=qwT, kwT=kwT, vwT=vwT, owT=owT,
            qtab=qtab, ktab=ktab, iden=iden_np, dm=dm))

    global _LAST_IN_MAPS
    _LAST_IN_MAPS = in_maps
    nc = _get_nc()
    res = run_bass_kernel_spmd(nc, in_maps, core_ids=list(range(8)))

    out = np.zeros((B, S, HID), np.float32)
    for c in range(8):
        b, hh, sh = c >> 2, (c >> 1) & 1, c & 1
        o = res.results[c]['out']  # [1024, 1024]
        for bi, g in enumerate(MYBLKS[sh]):
            out[b, g * 128:(g + 1) * 128, hh * 1024:(hh + 1) * 1024] = \
                o[bi * 128:(bi + 1) * 128]
    return out


if __name__ == '__main__':
    sys.path.insert(0, '/root/problem')
    import reference
    inputs = {k: np.asarray(v) for k, v in reference.setup_inputs().items()}
    exp = np.asarray(reference.reference(**inputs))
    act = kernel(**inputs)
    err = np.abs(act - exp)
    rel = np.linalg.norm(act - exp) / np.linalg.norm(exp)
    print('Relative error:', rel, 'max abs err:', err.max())


# revision 21
# speedup vs baseline: 1.0816x; 1.0816x over previous
"""Trainium2 Bass kernel for Qwen-style GQA attention block (B=2,S=2048,H=16,KV=8,D=128).

Sharding (8 cores): batch(2) x si-stripes(2) x head-half(2).
  core c: b=c>>2, sh=(c>>1)&1, hh=c&1
  - each core projects Q/K/V for ITS stripe rows only (1024 tokens); K/V results
    (roped, transposed, normalized) are exchanged between the two stripe cores
    via a small AllGather so both see full-S K/V.
  - attention j-outer with wide score tiles (stationary K-block reuse, wide exp).
  - pair AllGather of ctx^T split in two head-groups, column-split o_proj in two
    passes so the second collective hides under the first o_proj pass.
All matmuls bf16 with fp32 PSUM accumulation. Softmax without max-subtraction
(scores are O(1) after QK RMSNorm); denominator via an appended ones-column on V.
"""
import sys

sys.path.insert(0, '/opt/trn_rl_repo')

import numpy as np

import concourse.bass as bass
import concourse.tile as tile
from concourse import mybir
from concourse.vector_clock import ScopedClock, VectorClock

B, S, HID = 2, 2048, 2048
H, KV, D = 16, 8, 128
EPS = 1e-6
SCALE = D ** -0.5
NBLK = S // 128  # 16
# causally balanced si-block stripes: sum(i+1) = 68 for both
MYBLKS = [[0, 2, 4, 6, 9, 11, 13, 15], [1, 3, 5, 7, 8, 10, 12, 14]]
BOUNDS = [max(MYBLKS[0][bi], MYBLKS[1][bi]) for bi in range(8)]  # [1,3,..,15]
# o_proj ctx row order after the two pair-AllGathers (global head ids)
OHEAD_ORDER = [0, 1, 2, 3, 8, 9, 10, 11, 4, 5, 6, 7, 12, 13, 14, 15]

F32 = mybir.dt.float32
BF16 = mybir.dt.bfloat16
AF = mybir.ActivationFunctionType
MUL = mybir.AluOpType.mult
ADD = mybir.AluOpType.add


# ---------------------------------------------------------------------------
# Workarounds: this walrus supports only ONE sync-wait per instruction.
def _patched_drain_and_barrier(self, tick_clock, wait_clock):
    gc = tick_clock.global_clock
    vec = list(gc)
    nz = [i for i, v in enumerate(vec) if v > 0] or [0]
    for i in nz:
        cvec = [vec[j] if j == i else 0 for j in range(len(vec))]
        inst = self.nc.sync.drain()
        wait_clock.add_sem_waits(inst.ins, ScopedClock({None: VectorClock(cvec)}))
    self.nc.all_engine_barrier()
    assert self.sems is not None
    popped = self.nc._tile_sem_poison_stack.pop()
    assert popped is self._sem_poison
    self.nc.clear_and_free_semaphores(list(self.sems.allocated().values()))
    self.nc.all_engine_barrier()


tile.TileContext._drain_and_barrier = _patched_drain_and_barrier


def split_multi_waits(nc):
    for fn in nc.m.functions:
        for blk in fn.blocks:
            insts = list(blk.instructions)
            out = []
            changed = False
            for inst in insts:
                si = inst.sync_info
                if si is not None and len(si.on_wait) > 1:
                    waits = list(si.on_wait)
                    for k, w in enumerate(waits[:-1]):
                        out.append(mybir.InstNoOp(
                            name=f"{inst.name}.w{k}", engine=inst.engine,
                            sync_info=mybir.SyncInfo(on_wait=[w], on_update=[]),
                            text_hint="waitsplit"))
                    si.on_wait = [waits[-1]]
                    changed = True
                out.append(inst)
            if changed:
                blk.instructions[:] = out


# ---------------------------------------------------------------------------
def build_kernel():
    nc = bass.Bass(trn_type='TRN2')
    # hidden^T for THIS core's stripe rows (local bi-block order)
    hT = nc.dram_tensor('hT', [HID, 1024], F32, kind='ExternalInput')
    qwT = nc.dram_tensor('qwT', [HID, 1024], F32, kind='ExternalInput')
    kwT = nc.dram_tensor('kwT', [HID, 512], F32, kind='ExternalInput')
    vwT = nc.dram_tensor('vwT', [HID, 512], F32, kind='ExternalInput')
    owT = nc.dram_tensor('owT', [2048, 1024], F32, kind='ExternalInput')
    # host-fused rope tables (cos/sin x norm-weight halves), [1024, 4, 64]
    qtab = nc.dram_tensor('qtab', [1024, 4, 64], F32, kind='ExternalInput')
    ktab = nc.dram_tensor('ktab', [1024, 4, 64], F32, kind='ExternalInput')
    iden = nc.dram_tensor('iden', [128, 128], F32, kind='ExternalInput')
    # per-core diagonal masks dm[bi, t] for j in {BOUNDS[bi]-1, BOUNDS[bi]}
    dm = nc.dram_tensor('dm', [8, 2, 128, 128], F32, kind='ExternalInput')
    out_e = nc.dram_tensor('out', [1024, 1024], F32, kind='ExternalOutput')

    from contextlib import ExitStack
    with ExitStack() as ctx:
        tc = ctx.enter_context(tile.TileContext(nc))
        pool = lambda name, bufs, **kw: ctx.enter_context(
            tc.tile_pool(name=name, bufs=bufs, **kw))
        p_c = pool('const', 1)
        p_wv = pool('wv', 4)      # wv tiles, later QT tiles
        p_wk = pool('wk', 4)      # wk tiles, later ctm tiles
        p_wq = pool('wq', 4)      # wq tiles, later wo tiles
        p_h = pool('ht', 4)       # hT tiles, later ctf tiles
        p_kv = pool('kvstage', 1)  # KTh/VAh staging + KT_all/VA_all
        p_w = pool('work', 2)     # rope scratch etc
        p_s = pool('small', 4)
        p_ex = pool('expb', 4)
        p_ob = pool('outb', 2)
        ps_m = pool('psM', 3, space='PSUM')   # proj psums + scores + o_proj
        ps_c = pool('psC', 1, space='PSUM')   # ctx accumulators (3 tags)
        ps_t = pool('psT', 2, space='PSUM')   # transposes
        p_d = pool('dram', 1, space='DRAM')

        # ---- DMA prologue (gpsimd queue): wv/hT first so V proj starts early
        wv_s = [p_wv.tile([128, 4, 512], BF16, tag='wv', name='wv') for _ in range(4)]
        wk_s = [p_wk.tile([128, 4, 512], BF16, tag='wk', name='wk') for _ in range(4)]
        wq_s = [p_wq.tile([128, 4, 1024], BF16, tag='wq', name='wq') for _ in range(4)]
        ht_t = [p_h.tile([128, 4, 1024], BF16, tag='ht', name='ht') for _ in range(4)]
        for g in range(4):
            r = bass.ts(g, 512)
            nc.gpsimd.dma_start(wv_s[g][:], vwT[r, :].rearrange('(n p) c -> p n c', p=128))
            nc.gpsimd.dma_start(ht_t[g][:], hT[r, :].rearrange('(n p) c -> p n c', p=128))
        for g in range(4):
            nc.gpsimd.dma_start(wk_s[g][:], kwT[bass.ts(g, 512), :].rearrange('(n p) c -> p n c', p=128))
        iden_s = p_c.tile([128, 128], BF16)
        nc.gpsimd.dma_start(iden_s[:], iden[:])
        ktab_s = p_c.tile([128, 8, 4, 64], BF16)
        nc.gpsimd.dma_start(ktab_s[:], ktab.rearrange('(n p) t d -> p n t d', p=128))
        qtab_s = p_c.tile([128, 8, 4, 64], BF16)
        nc.gpsimd.dma_start(qtab_s[:], qtab.rearrange('(n p) t d -> p n t d', p=128))
        for g in range(4):
            nc.gpsimd.dma_start(wq_s[g][:], qwT[bass.ts(g, 512), :].rearrange('(n p) c -> p n c', p=128))
        dm_s = p_c.tile([128, 8, 2, 128], BF16)
        nc.gpsimd.dma_start(dm_s[:], dm.rearrange('n t p d -> p n t d'))

        # persistent K/V stores (full S, post-exchange) + local staging
        KTh = p_kv.tile([128, 4, 1024], BF16, tag='kth', name='KTh')
        VAh = p_kv.tile([128, 4, 8, 132], BF16, tag='vah', name='VAh')
        KT = p_kv.tile([128, 4, 2, 1024], BF16, tag='kt', name='KT')
        VA = p_kv.tile([128, 4, 16, 132], BF16, tag='va', name='VA')
        nc.gpsimd.memset(VAh[:, :, :, 128:132], 1.0)

        # ---- V projection: ch-major waves so PE starts on the first DMA chunk
        wave_tags = ['ps', 'ps', 'ps', 'c0', 'c1', 'c2']
        for wave, sbs in ((0, range(0, 6)), (1, range(6, 8))):
            psVs = {}
            for idx, sb in enumerate(sbs):
                tag = wave_tags[idx] if wave == 0 else 'ps'
                pool_ = ps_m if tag == 'ps' else ps_c
                psVs[sb] = pool_.tile([128, 512], F32, tag=tag, name=f'psV{sb}')
            for g in range(4):
                for i in range(4):
                    ch = g * 4 + i
                    for sb in sbs:
                        nc.tensor.matmul(psVs[sb][:], ht_t[g][:, i, bass.ts(sb, 128)],
                                         wv_s[g][:, i, :],
                                         start=(ch == 0), stop=(ch == 15))
            for sb in sbs:
                nc.scalar.copy(VAh[:, :, sb, 0:128],
                               psVs[sb][:].rearrange('p (k d) -> p k d', k=4))

        # ---- K projection + RMSNorm(*SCALE) + rope + transpose ----
        for sb in range(8):
            psK = ps_m.tile([128, 512], F32, tag='ps', name='psK')
            for ch in range(16):
                nc.tensor.matmul(psK[:], ht_t[ch // 4][:, ch % 4, bass.ts(sb, 128)],
                                 wk_s[ch // 4][:, ch % 4, :],
                                 start=(ch == 0), stop=(ch == 15))
            kraw = p_w.tile([128, 4, 128], BF16, tag='raw', name='kraw')
            nc.scalar.copy(kraw[:], psK[:].rearrange('p (k d) -> p k d', k=4))
            sqd = p_w.tile([128, 4, 128], BF16, tag='sqd', name='sqd')
            nc.vector.tensor_mul(sqd[:], kraw[:], kraw[:])
            ms = p_s.tile([128, 4], F32, tag='ms', name='ms')
            nc.vector.tensor_reduce(ms[:], sqd[:], mybir.AxisListType.X, ADD)
            nc.vector.tensor_scalar_add(ms[:], ms[:], float(EPS * D))
            std = p_s.tile([128, 4], F32, tag='std', name='std')
            nc.scalar.activation(std[:], ms[:], AF.Sqrt, scale=1.0 / D, bias=0.0)
            rstd = p_s.tile([128, 4], F32, tag='rstd', name='rstd')
            nc.vector.reciprocal(rstd[:], std[:])
            rstdS = p_s.tile([128, 4], F32, tag='rstds', name='rstdS')
            nc.vector.tensor_scalar_mul(rstdS[:], rstd[:], SCALE)
            kcs = p_w.tile([128, 4, 128], BF16, tag='kcs', name='kcs')
            for kvh in range(4):
                nc.vector.tensor_scalar_mul(kcs[:, kvh, :], kraw[:, kvh, :],
                                            rstdS[:, kvh:kvh + 1])
            # rope on gpsimd (keeps DVE free); tables already fold k_norm_w
            lo, hi = kcs[:, :, 0:64], kcs[:, :, 64:128]
            tA = ktab_s[:, sb, :, :][:, 0:1, :]
            tB = ktab_s[:, sb, :, :][:, 1:2, :]
            tC = ktab_s[:, sb, :, :][:, 2:3, :]
            tD = ktab_s[:, sb, :, :][:, 3:4, :]
            t_ = p_w.tile([128, 4, 4, 64], BF16, tag='t4', name='t4')
            kro = p_w.tile([128, 4, 128], BF16, tag='kro', name='kro')
            mul_b(nc.gpsimd, t_[:, 0], lo, tA)
            mul_b(nc.gpsimd, t_[:, 1], hi, tB)
            nc.gpsimd.tensor_sub(kro[:, :, 0:64], t_[:, 0], t_[:, 1])
            mul_b(nc.gpsimd, t_[:, 2], hi, tC)
            mul_b(nc.gpsimd, t_[:, 3], lo, tD)
            nc.gpsimd.tensor_add(kro[:, :, 64:128], t_[:, 2], t_[:, 3])
            psTk = ps_t.tile([128, 4, 128], BF16, tag='pst', name='psTk')
            for kvh in range(4):
                nc.tensor.transpose(psTk[:, kvh, :], kro[:, kvh, :], iden_s[:])
            nc.scalar.copy(KTh[:, :, bass.ts(sb, 128)], psTk[:])

        # ---- exchange K/V halves between the stripe pair (hidden under Q) ----
        # >=512-row DRAM shapes so the runtime picks the fast Mesh algorithm
        ccK_in = p_d.tile([512, 1024], BF16, tag='cki', name='ccK_in')
        ccK_out = p_d.tile([1024, 1024], BF16, tag='cko', name='ccK_out')
        ccV_in = p_d.tile([512, 1056], BF16, tag='cvi', name='ccV_in')
        ccV_out = p_d.tile([1024, 1056], BF16, tag='cvo', name='ccV_out')
        nc.sync.dma_start(ccK_in[:].rearrange('(p c) s -> p c s', p=128), KTh[:])
        nc.sync.dma_start(ccV_in[:].rearrange('(p c) s -> p c s', p=128),
                          VAh[:].rearrange('p k s w -> p k (s w)'))
        for ci, co in ((ccK_in, ccK_out), (ccV_in, ccV_out)):
            nc.gpsimd.collective_compute(
                'AllGather', mybir.AluOpType.bypass,
                replica_groups=[[0, 1], [2, 3], [4, 5], [6, 7]],
                ins=[ci.opt()], outs=[co.opt()])
        # read back both stripes, kept in stripe-local order:
        # global block j lives at (rank r_j, slot j//2), r_j = (j%2) ^ (j>=8)
        for r in range(2):
            nc.sync.dma_start(
                KT[:, :, r, :],
                ccK_out[bass.ts(r, 512), :].rearrange('(p c) s -> p c s', p=128))
            nc.sync.dma_start(
                VA[:, :, r * 8:(r + 1) * 8, :].rearrange('p k s w -> p k (s w)'),
                ccV_out[bass.ts(r, 512), :].rearrange('(p c) s -> p c s', p=128))

        # ---- Q projection (PE busy while exchange completes) ----
        QT = [p_wv.tile([128, 2, 1024], BF16, tag='wv', name='QT') for _ in range(4)]
        for bi in range(8):
            for qg in range(2):
                psQ = ps_m.tile([128, 512], F32, tag='ps', name='psQ')
                for ch in range(16):
                    nc.tensor.matmul(psQ[:], ht_t[ch // 4][:, ch % 4, bass.ts(bi, 128)],
                                     wq_s[ch // 4][:, ch % 4, bass.ts(qg, 512)],
                                     start=(ch == 0), stop=(ch == 15))
                qraw = p_w.tile([128, 4, 128], BF16, tag='raw', name='qraw')
                nc.scalar.copy(qraw[:], psQ[:].rearrange('p (k d) -> p k d', k=4))
                sqd = p_w.tile([128, 4, 128], BF16, tag='sqd', name='sqd')
                nc.vector.tensor_mul(sqd[:], qraw[:], qraw[:])
                ms = p_s.tile([128, 4], F32, tag='ms', name='ms')
                nc.vector.tensor_reduce(ms[:], sqd[:], mybir.AxisListType.X, ADD)
                nc.vector.tensor_scalar_add(ms[:], ms[:], float(EPS * D))
                std = p_s.tile([128, 4], F32, tag='std', name='std')
                nc.scalar.activation(std[:], ms[:], AF.Sqrt, scale=1.0 / D, bias=0.0)
                rstd = p_s.tile([128, 4], F32, tag='rstd', name='rstd')
                nc.vector.reciprocal(rstd[:], std[:])
                qcs = p_w.tile([128, 4, 128], BF16, tag='kcs', name='qcs')
                for hq in range(4):
                    nc.vector.tensor_scalar_mul(qcs[:, hq, :], qraw[:, hq, :],
                                                rstd[:, hq:hq + 1])
                lo, hi = qcs[:, :, 0:64], qcs[:, :, 64:128]
                tA = qtab_s[:, bi, :, :][:, 0:1, :]
                tB = qtab_s[:, bi, :, :][:, 1:2, :]
                tC = qtab_s[:, bi, :, :][:, 2:3, :]
                tD = qtab_s[:, bi, :, :][:, 3:4, :]
                t_ = p_w.tile([128, 4, 4, 64], BF16, tag='t4', name='t4')
                qro = p_w.tile([128, 4, 128], BF16, tag='kro', name='qro')
                eng = nc.vector if qg == 0 else nc.gpsimd
                mul_b(eng, t_[:, 0], lo, tA)
                mul_b(eng, t_[:, 1], hi, tB)
                eng.tensor_sub(qro[:, :, 0:64], t_[:, 0], t_[:, 1])
                mul_b(eng, t_[:, 2], hi, tC)
                mul_b(eng, t_[:, 3], lo, tD)
                eng.tensor_add(qro[:, :, 64:128], t_[:, 2], t_[:, 3])
                psTq = ps_t.tile([128, 4, 128], BF16, tag='pst', name='psTq')
                for hq in range(4):
                    nc.tensor.transpose(psTq[:, hq, :], qro[:, hq, :], iden_s[:])
                nc.scalar.copy(QT[qg * 2][:, :, bass.ts(bi, 128)], psTq[:, 0:2, :])
                nc.scalar.copy(QT[qg * 2 + 1][:, :, bass.ts(bi, 128)], psTq[:, 2:4, :])

        # wo loads (reuse wq slots; runs during attention)
        wo_s = [p_wq.tile([128, 4, 1024], BF16, tag='wq', name='wo') for _ in range(4)]
        for g in range(4):
            nc.gpsimd.dma_start(wo_s[g][:], owT[bass.ts(g, 512), :].rearrange('(n p) c -> p n c', p=128))

        # ---- attention: j-outer, wide score tiles ----
        ctm = [p_wk.tile([128, 2, 1024], BF16, tag='wk', name='ctm') for _ in range(4)]
        # psC slot layout: tag -> (first bi, n slots)
        CGRP = [(0, 3), (3, 3), (6, 2)]

        def cslot(pc, bi):
            g = 0 if bi < 3 else (1 if bi < 6 else 2)
            s = bi - CGRP[g][0]
            return pc[g][:, s * 132:s * 132 + 129]

        ctfs = []
        for h in range(8):
            kvh = h // 2
            qt_ap = QT[h // 2][:, h % 2, :]
            pc = [ps_c.tile([128, CGRP[g][1] * 132], F32, tag=f'c{g}', name=f'pc{g}')
                  for g in range(3)]
            exs = {}
            psTc = None
            for it in range(17):
                # QK for j=it (chunks), exp, mask
                if it < 16:
                    j = it
                    b0 = j // 2
                    if b0 < 4:
                        chunks = [(b0 * 128, 512), (512, 1024)]
                    else:
                        chunks = [(b0 * 128, 1024)]
                    rj = (j % 2) ^ (1 if j >= 8 else 0)
                    kt_ap = KT[:, kvh, rj, bass.ts(j // 2, 128)]
                    cur = []
                    for (c0, c1) in chunks:
                        w = c1 - c0
                        psS = ps_m.tile([128, 512], F32, tag='ps', name='psS')
                        nc.tensor.matmul(psS[:, 0:w], kt_ap, qt_ap[:, c0:c1],
                                         start=True, stop=True)
                        ex = p_ex.tile([128, 512], BF16, tag='ex', name='ex')
                        nc.scalar.activation(ex[:, 0:w], psS[:, 0:w], AF.Exp)
                        cur.append((c0, c1, ex))
                    # diagonal/overhang mask: block bi=b0, t = j parity
                    nc.vector.tensor_mul(cur[0][2][:, 0:128], cur[0][2][:, 0:128],
                                         dm_s[:, b0, j % 2, :])
                    exs[j] = cur
                # PV for j=it-1 (+finalizations)
                if it > 0:
                    j = it - 1
                    b0 = j // 2
                    rj = (j % 2) ^ (1 if j >= 8 else 0)
                    va_ap = VA[:, kvh, rj * 8 + j // 2, 0:129]
                    for bi in range(b0, 8):
                        (c0, c1, ex) = exs[j][0] if bi * 128 < exs[j][0][1] else exs[j][1]
                        exsub = ex[:, bi * 128 - c0:bi * 128 - c0 + 128]
                        # start=True clears has_written for the WHOLE bank: issue it
                        # only on the bank's first matmul; other slots first-write
                        # via the overwrite-where-unwritten path.
                        nc.tensor.matmul(cslot(pc, bi), exsub, va_ap,
                                         start=(j == 0 and bi in (0, 3, 6)),
                                         stop=(j == BOUNDS[bi]),
                                         skip_group_check=True)
                    del exs[j]
                    # finalize bi whose last block was j
                    if it % 2 == 0:
                        bi = (it - 2) // 2
                        sl = cslot(pc, bi)
                        rd = p_s.tile([128, 1], F32, tag='rd', name='rd')
                        nc.vector.reciprocal(rd[:], sl[:, 128:129])
                        cn = p_w.tile([128, 128], BF16, tag='cn', name='cn')
                        nc.vector.tensor_scalar_mul(cn[:], sl[:, 0:128], rd[:])
                        if psTc is None:
                            psTc = ps_t.tile([128, 4, 128], BF16, tag='pst', name='psTc')
                        nc.tensor.transpose(psTc[:, bi % 2, :], cn[:], iden_s[:])
                        if bi % 2 == 1:
                            nc.vector.tensor_scalar_add(
                                ctm[h // 2][:, h % 2, bass.ts(bi // 2, 256)],
                                psTc[:, 0:2, :], 0.0)
                            psTc = None
            # ctx pair-AllGather in 2 chunks (after h3/h7); 512-row shapes -> Mesh
            if h % 4 == 3:
                p = h // 4
                cci = p_d.tile([512, 1024], BF16, tag=f'cci{p}', name=f'cci{p}')
                cco = p_d.tile([1024, 1024], BF16, tag=f'cco{p}', name=f'cco{p}')
                for i in range(2):
                    nc.sync.dma_start(
                        cci[bass.ts(i, 256), :].rearrange('(c p) s -> p c s', c=2),
                        ctm[2 * p + i][:])
                nc.gpsimd.collective_compute(
                    'AllGather', mybir.AluOpType.bypass,
                    replica_groups=[[0, 2], [1, 3], [4, 6], [5, 7]],
                    ins=[cci.opt()], outs=[cco.opt()])
                pool_f, tag_f = (p_h, 'ht') if p == 0 else (p_wv, 'wv')
                cf = [pool_f.tile([128, 2, 1024], BF16, tag=tag_f, name=f'ctf{p}')
                      for _ in range(4)]
                for i in range(4):
                    nc.sync.dma_start(
                        cf[i][:],
                        cco[bass.ts(i, 256), :].rearrange('(c p) s -> p c s', c=2))
                ctfs.append(cf)

        # ---- o_proj: 2 passes (one per ctx chunk), accumulate in SBUF ----
        obuf = p_kv.tile([128, 8, 1024], BF16, tag='obuf', name='obuf')
        for p in range(2):
            cf = ctfs[p]
            for bi in range(8):
                for nt in range(2):
                    psO = ps_m.tile([128, 512], F32, tag='ps', name='psO')
                    for c in range(8):
                        nc.tensor.matmul(psO[:], cf[c // 2][:, c % 2, bass.ts(bi, 128)],
                                         wo_s[2 * p + c // 4][:, c % 4, bass.ts(nt, 512)],
                                         start=(c == 0), stop=(c == 7))
                    dst = obuf[:, bi, bass.ts(nt, 512)]
                    if p == 0:
                        nc.vector.tensor_scalar_add(dst, psO[:], 0.0)
                    else:
                        ob = p_ob.tile([128, 512], F32, tag='ob', name='ob')
                        nc.vector.tensor_add(ob[:], psO[:], dst)
                        nc.sync.dma_start(out_e[bass.ts(bi, 128), bass.ts(nt, 512)], ob[:])

    split_multi_waits(nc)
    return nc


def mul_b(eng, out, a, b):
    """tensor_tensor multiply with free-dim broadcast of b over dim 1."""
    a2, b2 = bass.broadcast_tensor_aps(a, b)
    eng.tensor_mul(out, a2, b2)


# ---------------------------------------------------------------------------
_NC_CACHE = None
_LAST_IN_MAPS = None


def _get_nc():
    global _NC_CACHE
    if _NC_CACHE is None:
        _NC_CACHE = build_kernel()
    return _NC_CACHE


def kernel(hidden_states, cos, sin, q_w, k_w, v_w, o_w, q_norm_w, k_norm_w):
    from concourse.bass_utils import run_bass_kernel_spmd

    hidden_states = np.asarray(hidden_states, np.float32)
    cos = np.asarray(cos, np.float32)
    sin = np.asarray(sin, np.float32)
    q_w = np.asarray(q_w, np.float32)
    k_w = np.asarray(k_w, np.float32)
    v_w = np.asarray(v_w, np.float32)
    o_w = np.asarray(o_w, np.float32)
    q_norm_w = np.asarray(q_norm_w, np.float32)
    k_norm_w = np.asarray(k_norm_w, np.float32)

    tri_np = np.triu(np.ones((128, 128), np.float32))  # [sj,si]: valid sj<=si
    iden_np = np.eye(128, dtype=np.float32)
    operm = np.concatenate([np.arange(h * 128, (h + 1) * 128) for h in OHEAD_ORDER])

    def rope_tabs(c, s_, w):
        # tables [rows, 4, 64]: A=c_lo*w_lo, B=s_lo*w_hi, C=c_lo*w_hi, D=s_lo*w_lo
        cl, sl = c[:, 0:64], s_[:, 0:64]
        wl, wh = w[0:64], w[64:128]
        return np.stack([cl * wl, sl * wh, cl * wh, sl * wl], axis=1).astype(np.float32)

    in_maps = []
    for c in range(8):
        b, hh, sh = c >> 2, (c >> 1) & 1, c & 1
        blks = MYBLKS[sh]
        rows = np.concatenate([np.arange(g * 128, (g + 1) * 128) for g in blks])
        hT = np.ascontiguousarray(hidden_states[b][rows].T)
        qwT = np.ascontiguousarray(q_w[hh * 1024:(hh + 1) * 1024].T)
        kwT = np.ascontiguousarray(k_w[hh * 512:(hh + 1) * 512].T)
        vwT = np.ascontiguousarray(v_w[hh * 512:(hh + 1) * 512].T)
        owT = np.ascontiguousarray(o_w[hh * 1024:(hh + 1) * 1024].T[operm])
        qtab = rope_tabs(cos[b][rows], sin[b][rows], q_norm_w)
        ktab = rope_tabs(cos[b][rows], sin[b][rows], k_norm_w)
        # diagonal masks dm[bi, t]: t=0 -> sj block BOUNDS[bi]-1, t=1 -> BOUNDS[bi]
        dm = np.zeros((8, 2, 128, 128), np.float32)
        for bi in range(8):
            g, gb = blks[bi], BOUNDS[bi]
            for t, j in enumerate((gb - 1, gb)):
                if j < g:
                    dm[bi, t] = 1.0
                elif j == g:
                    dm[bi, t] = tri_np
                # j > g: stays 0 (block fully masked)
        in_maps.append(dict(
            hT=hT, qwT=qwT, kwT=kwT, vwT=vwT, owT=owT,
            qtab=qtab, ktab=ktab, iden=iden_np, dm=dm))

    global _LAST_IN_MAPS
    _LAST_IN_MAPS = in_maps
    nc = _get_nc()
    res = run_bass_kernel_spmd(nc, in_maps, core_ids=list(range(8)))

    out = np.zeros((B, S, HID), np.float32)
    for c in range(8):
        b, hh, sh = c >> 2, (c >> 1) & 1, c & 1
        o = res.results[c]['out']  # [1024, 1024]
        for bi, g in enumerate(MYBLKS[sh]):
            out[b, g * 128:(g + 1) * 128, hh * 1024:(hh + 1) * 1024] = \
                o[bi * 128:(bi + 1) * 128]
    return out


if __name__ == '__main__':
    sys.path.insert(0, '/root/problem')
    import reference
    inputs = {k: np.asarray(v) for k, v in reference.setup_inputs().items()}
    exp = np.asarray(reference.reference(**inputs))
    act = kernel(**inputs)
    err = np.abs(act - exp)
    rel = np.linalg.norm(act - exp) / np.linalg.norm(exp)
    print('Relative error:', rel, 'max abs err:', err.max())


# revision 24
# speedup vs baseline: 1.1907x; 1.1009x over previous
"""Trainium2 Bass kernel for Qwen-style GQA attention block (B=2,S=2048,H=16,KV=8,D=128).

Sharding (8 cores): batch(2) x si-stripes(2) x head-half(2).
  core c: b=c>>2, sh=(c>>1)&1, hh=c&1
  - each core projects Q/K/V for ITS stripe rows only (1024 tokens); K/V results
    (roped, transposed, normalized) are exchanged between the two stripe cores
    via a small AllGather so both see full-S K/V.
  - attention j-outer with wide score tiles (stationary K-block reuse, wide exp).
  - pair AllGather of ctx^T split in two head-groups, column-split o_proj in two
    passes so the second collective hides under the first o_proj pass.
All matmuls bf16 with fp32 PSUM accumulation. Softmax without max-subtraction
(scores are O(1) after QK RMSNorm); denominator via an appended ones-column on V.
"""
import sys

sys.path.insert(0, '/opt/trn_rl_repo')

import numpy as np
import ml_dtypes

import concourse.bass as bass
import concourse.tile as tile
from concourse import mybir
from concourse.vector_clock import ScopedClock, VectorClock

B, S, HID = 2, 2048, 2048
H, KV, D = 16, 8, 128
EPS = 1e-6
SCALE = D ** -0.5
NBLK = S // 128  # 16
# causally balanced si-block stripes: sum(i+1) = 68 for both
MYBLKS = [[0, 2, 4, 6, 9, 11, 13, 15], [1, 3, 5, 7, 8, 10, 12, 14]]
BOUNDS = [max(MYBLKS[0][bi], MYBLKS[1][bi]) for bi in range(8)]  # [1,3,..,15]
# o_proj ctx row order after the two pair-AllGathers (global head ids)
OHEAD_ORDER = [0, 1, 2, 3, 8, 9, 10, 11, 4, 5, 6, 7, 12, 13, 14, 15]

F32 = mybir.dt.float32
BF16 = mybir.dt.bfloat16
AF = mybir.ActivationFunctionType
MUL = mybir.AluOpType.mult
ADD = mybir.AluOpType.add


# ---------------------------------------------------------------------------
# Workarounds: this walrus supports only ONE sync-wait per instruction.
def _patched_drain_and_barrier(self, tick_clock, wait_clock):
    gc = tick_clock.global_clock
    vec = list(gc)
    nz = [i for i, v in enumerate(vec) if v > 0] or [0]
    for i in nz:
        cvec = [vec[j] if j == i else 0 for j in range(len(vec))]
        inst = self.nc.sync.drain()
        wait_clock.add_sem_waits(inst.ins, ScopedClock({None: VectorClock(cvec)}))
    self.nc.all_engine_barrier()
    assert self.sems is not None
    popped = self.nc._tile_sem_poison_stack.pop()
    assert popped is self._sem_poison
    self.nc.clear_and_free_semaphores(list(self.sems.allocated().values()))
    self.nc.all_engine_barrier()


tile.TileContext._drain_and_barrier = _patched_drain_and_barrier


def split_multi_waits(nc):
    for fn in nc.m.functions:
        for blk in fn.blocks:
            insts = list(blk.instructions)
            out = []
            changed = False
            for inst in insts:
                si = inst.sync_info
                if si is not None and len(si.on_wait) > 1:
                    waits = list(si.on_wait)
                    for k, w in enumerate(waits[:-1]):
                        out.append(mybir.InstNoOp(
                            name=f"{inst.name}.w{k}", engine=inst.engine,
                            sync_info=mybir.SyncInfo(on_wait=[w], on_update=[]),
                            text_hint="waitsplit"))
                    si.on_wait = [waits[-1]]
                    changed = True
                out.append(inst)
            if changed:
                blk.instructions[:] = out


# ---------------------------------------------------------------------------
def build_kernel():
    nc = bass.Bass(trn_type='TRN2')
    # hidden^T for THIS core's stripe rows (local bi-block order)
    hT = nc.dram_tensor('hT', [HID, 1024], BF16, kind='ExternalInput')
    qwT = nc.dram_tensor('qwT', [HID, 1024], BF16, kind='ExternalInput')
    kwT = nc.dram_tensor('kwT', [HID, 512], BF16, kind='ExternalInput')
    vwT = nc.dram_tensor('vwT', [HID, 512], BF16, kind='ExternalInput')
    owT = nc.dram_tensor('owT', [2048, 1024], BF16, kind='ExternalInput')
    # host-fused rope tables (cos/sin x norm-weight halves), [1024, 4, 64]
    qtab = nc.dram_tensor('qtab', [1024, 4, 64], BF16, kind='ExternalInput')
    ktab = nc.dram_tensor('ktab', [1024, 4, 64], BF16, kind='ExternalInput')
    iden = nc.dram_tensor('iden', [128, 128], BF16, kind='ExternalInput')
    # per-core diagonal masks dm[bi, t] for j in {BOUNDS[bi]-1, BOUNDS[bi]}
    dm = nc.dram_tensor('dm', [8, 2, 128, 128], BF16, kind='ExternalInput')
    out_e = nc.dram_tensor('out', [1024, 1024], F32, kind='ExternalOutput')

    from contextlib import ExitStack
    with ExitStack() as ctx:
        tc = ctx.enter_context(tile.TileContext(nc))
        pool = lambda name, bufs, **kw: ctx.enter_context(
            tc.tile_pool(name=name, bufs=bufs, **kw))
        p_c = pool('const', 1)
        p_wv = pool('wv', 4)      # wv tiles, later QT tiles
        p_wk = pool('wk', 4)      # wk tiles, later ctm tiles
        p_wq = pool('wq', 4)      # wq tiles, later wo tiles
        p_h = pool('ht', 4)       # hT tiles, later ctf tiles
        p_kv = pool('kvstage', 1)  # KTh/VAh staging + KT_all/VA_all
        p_w = pool('work', 2)     # rope scratch etc
        p_s = pool('small', 4)
        p_ex = pool('expb', 4)
        p_ob = pool('outb', 2)
        ps_m = pool('psM', 3, space='PSUM')   # proj psums + scores + o_proj
        ps_c = pool('psC', 1, space='PSUM')   # ctx accumulators (3 tags)
        ps_t = pool('psT', 2, space='PSUM')   # transposes
        p_d = pool('dram', 1, space='DRAM')

        # ---- DMA prologue (gpsimd queue): wv/hT first so V proj starts early
        wv_s = [p_wv.tile([128, 4, 512], BF16, tag='wv', name='wv') for _ in range(4)]
        wk_s = [p_wk.tile([128, 4, 512], BF16, tag='wk', name='wk') for _ in range(4)]
        wq_s = [p_wq.tile([128, 4, 1024], BF16, tag='wq', name='wq') for _ in range(4)]
        ht_t = [p_h.tile([128, 4, 1024], BF16, tag='ht', name='ht') for _ in range(4)]
        for g in range(4):
            r = bass.ts(g, 512)
            nc.gpsimd.dma_start(wv_s[g][:], vwT[r, :].rearrange('(n p) c -> p n c', p=128))
            nc.gpsimd.dma_start(ht_t[g][:], hT[r, :].rearrange('(n p) c -> p n c', p=128))
        for g in range(4):
            nc.gpsimd.dma_start(wk_s[g][:], kwT[bass.ts(g, 512), :].rearrange('(n p) c -> p n c', p=128))
        iden_s = p_c.tile([128, 128], BF16)
        nc.gpsimd.dma_start(iden_s[:], iden[:])
        ktab_s = p_c.tile([128, 8, 4, 64], BF16)
        nc.gpsimd.dma_start(ktab_s[:], ktab.rearrange('(n p) t d -> p n t d', p=128))
        qtab_s = p_c.tile([128, 8, 4, 64], BF16)
        nc.gpsimd.dma_start(qtab_s[:], qtab.rearrange('(n p) t d -> p n t d', p=128))
        for g in range(4):
            nc.gpsimd.dma_start(wq_s[g][:], qwT[bass.ts(g, 512), :].rearrange('(n p) c -> p n c', p=128))
        dm_s = p_c.tile([128, 8, 2, 128], BF16)
        nc.gpsimd.dma_start(dm_s[:], dm.rearrange('n t p d -> p n t d'))

        # persistent K/V stores (full S, post-exchange) + local staging
        KTh = p_kv.tile([128, 4, 1024], BF16, tag='kth', name='KTh')
        VAh = p_kv.tile([128, 4, 8, 132], BF16, tag='vah', name='VAh')
        KT = p_kv.tile([128, 4, 2, 1024], BF16, tag='kt', name='KT')
        VA = p_kv.tile([128, 4, 16, 132], BF16, tag='va', name='VA')
        nc.gpsimd.memset(VAh[:, :, :, 128:132], 1.0)

        # ---- V projection: ch-major waves so PE starts on the first DMA chunk
        wave_tags = ['ps', 'ps', 'ps', 'c0', 'c1', 'c2']
        for wave, sbs in ((0, range(0, 6)), (1, range(6, 8))):
            psVs = {}
            for idx, sb in enumerate(sbs):
                tag = wave_tags[idx] if wave == 0 else 'ps'
                pool_ = ps_m if tag == 'ps' else ps_c
                psVs[sb] = pool_.tile([128, 512], F32, tag=tag, name=f'psV{sb}')
            for g in range(4):
                for i in range(4):
                    ch = g * 4 + i
                    for sb in sbs:
                        nc.tensor.matmul(psVs[sb][:], ht_t[g][:, i, bass.ts(sb, 128)],
                                         wv_s[g][:, i, :],
                                         start=(ch == 0), stop=(ch == 15))
            for sb in sbs:
                nc.scalar.copy(VAh[:, :, sb, 0:128],
                               psVs[sb][:].rearrange('p (k d) -> p k d', k=4))

        # ---- V exchange: launch immediately so even a slow Ring hides ----
        ccV_in = p_d.tile([512, 1056], BF16, tag='cvi', name='ccV_in')
        ccV_out = p_d.tile([1024, 1056], BF16, tag='cvo', name='ccV_out')
        nc.sync.dma_start(ccV_in[:].rearrange('(p c) s -> p c s', p=128),
                          VAh[:].rearrange('p k s w -> p k (s w)'))
        nc.gpsimd.collective_compute(
            'AllGather', mybir.AluOpType.bypass,
            replica_groups=[[0, 2], [1, 3], [4, 6], [5, 7]],
            ins=[ccV_in.opt()], outs=[ccV_out.opt()])
        for r in range(2):
            nc.sync.dma_start(
                VA[:, :, r * 8:(r + 1) * 8, :].rearrange('p k s w -> p k (s w)'),
                ccV_out[bass.ts(r, 512), :].rearrange('(p c) s -> p c s', p=128))

        # ---- K projection + RMSNorm(*SCALE) + rope + transpose ----
        for sb in range(8):
            psK = ps_m.tile([128, 512], F32, tag='ps', name='psK')
            for ch in range(16):
                nc.tensor.matmul(psK[:], ht_t[ch // 4][:, ch % 4, bass.ts(sb, 128)],
                                 wk_s[ch // 4][:, ch % 4, :],
                                 start=(ch == 0), stop=(ch == 15))
            kraw = p_w.tile([128, 4, 128], BF16, tag='raw', name='kraw')
            nc.scalar.copy(kraw[:], psK[:].rearrange('p (k d) -> p k d', k=4))
            sqd = p_w.tile([128, 4, 128], BF16, tag='sqd', name='sqd')
            nc.vector.tensor_mul(sqd[:], kraw[:], kraw[:])
            ms = p_s.tile([128, 4], F32, tag='ms', name='ms')
            nc.vector.tensor_reduce(ms[:], sqd[:], mybir.AxisListType.X, ADD)
            nc.vector.tensor_scalar_add(ms[:], ms[:], float(EPS * D))
            std = p_s.tile([128, 4], F32, tag='std', name='std')
            nc.scalar.activation(std[:], ms[:], AF.Sqrt, scale=1.0 / D, bias=0.0)
            rstd = p_s.tile([128, 4], F32, tag='rstd', name='rstd')
            nc.vector.reciprocal(rstd[:], std[:])
            rstdS = p_s.tile([128, 4], F32, tag='rstds', name='rstdS')
            nc.vector.tensor_scalar_mul(rstdS[:], rstd[:], SCALE)
            kcs = p_w.tile([128, 4, 128], BF16, tag='kcs', name='kcs')
            for kvh in range(4):
                nc.vector.tensor_scalar_mul(kcs[:, kvh, :], kraw[:, kvh, :],
                                            rstdS[:, kvh:kvh + 1])
            # rope on gpsimd (keeps DVE free); tables already fold k_norm_w
            lo, hi = kcs[:, :, 0:64], kcs[:, :, 64:128]
            tA = ktab_s[:, sb, :, :][:, 0:1, :]
            tB = ktab_s[:, sb, :, :][:, 1:2, :]
            tC = ktab_s[:, sb, :, :][:, 2:3, :]
            tD = ktab_s[:, sb, :, :][:, 3:4, :]
            t_ = p_w.tile([128, 4, 4, 64], BF16, tag='t4', name='t4')
            kro = p_w.tile([128, 4, 128], BF16, tag='kro', name='kro')
            mul_b(nc.gpsimd, t_[:, 0], lo, tA)
            mul_b(nc.gpsimd, t_[:, 1], hi, tB)
            nc.gpsimd.tensor_sub(kro[:, :, 0:64], t_[:, 0], t_[:, 1])
            mul_b(nc.gpsimd, t_[:, 2], hi, tC)
            mul_b(nc.gpsimd, t_[:, 3], lo, tD)
            nc.gpsimd.tensor_add(kro[:, :, 64:128], t_[:, 2], t_[:, 3])
            psTk = ps_t.tile([128, 4, 128], BF16, tag='pst', name='psTk')
            for kvh in range(4):
                nc.tensor.transpose(psTk[:, kvh, :], kro[:, kvh, :], iden_s[:])
            nc.scalar.copy(KTh[:, :, bass.ts(sb, 128)], psTk[:])

        # ---- K exchange (V already in flight). Stripe-local order store:
        # global block j lives at (rank r_j, slot j//2), r_j = (j%2) ^ (j>=8)
        ccK_in = p_d.tile([512, 1024], BF16, tag='cki', name='ccK_in')
        ccK_out = p_d.tile([1024, 1024], BF16, tag='cko', name='ccK_out')
        nc.sync.dma_start(ccK_in[:].rearrange('(p c) s -> p c s', p=128), KTh[:])
        nc.gpsimd.collective_compute(
            'AllGather', mybir.AluOpType.bypass,
            replica_groups=[[0, 2], [1, 3], [4, 6], [5, 7]],
            ins=[ccK_in.opt()], outs=[ccK_out.opt()])
        for r in range(2):
            nc.sync.dma_start(
                KT[:, :, r, :],
                ccK_out[bass.ts(r, 512), :].rearrange('(p c) s -> p c s', p=128))

        # ---- Q projection (PE busy while exchange completes) ----
        QT = [p_wv.tile([128, 2, 1024], BF16, tag='wv', name='QT') for _ in range(4)]
        for bi in range(8):
            for qg in range(2):
                psQ = ps_m.tile([128, 512], F32, tag='ps', name='psQ')
                for ch in range(16):
                    nc.tensor.matmul(psQ[:], ht_t[ch // 4][:, ch % 4, bass.ts(bi, 128)],
                                     wq_s[ch // 4][:, ch % 4, bass.ts(qg, 512)],
                                     start=(ch == 0), stop=(ch == 15))
                qraw = p_w.tile([128, 4, 128], BF16, tag='raw', name='qraw')
                nc.scalar.copy(qraw[:], psQ[:].rearrange('p (k d) -> p k d', k=4))
                sqd = p_w.tile([128, 4, 128], BF16, tag='sqd', name='sqd')
                nc.vector.tensor_mul(sqd[:], qraw[:], qraw[:])
                ms = p_s.tile([128, 4], F32, tag='ms', name='ms')
                nc.vector.tensor_reduce(ms[:], sqd[:], mybir.AxisListType.X, ADD)
                nc.vector.tensor_scalar_add(ms[:], ms[:], float(EPS * D))
                std = p_s.tile([128, 4], F32, tag='std', name='std')
                nc.scalar.activation(std[:], ms[:], AF.Sqrt, scale=1.0 / D, bias=0.0)
                rstd = p_s.tile([128, 4], F32, tag='rstd', name='rstd')
                nc.vector.reciprocal(rstd[:], std[:])
                qcs = p_w.tile([128, 4, 128], BF16, tag='kcs', name='qcs')
                for hq in range(4):
                    nc.vector.tensor_scalar_mul(qcs[:, hq, :], qraw[:, hq, :],
                                                rstd[:, hq:hq + 1])
                lo, hi = qcs[:, :, 0:64], qcs[:, :, 64:128]
                tA = qtab_s[:, bi, :, :][:, 0:1, :]
                tB = qtab_s[:, bi, :, :][:, 1:2, :]
                tC = qtab_s[:, bi, :, :][:, 2:3, :]
                tD = qtab_s[:, bi, :, :][:, 3:4, :]
                t_ = p_w.tile([128, 4, 4, 64], BF16, tag='t4', name='t4')
                qro = p_w.tile([128, 4, 128], BF16, tag='kro', name='qro')
                eng = nc.vector if qg == 0 else nc.gpsimd
                mul_b(eng, t_[:, 0], lo, tA)
                mul_b(eng, t_[:, 1], hi, tB)
                eng.tensor_sub(qro[:, :, 0:64], t_[:, 0], t_[:, 1])
                mul_b(eng, t_[:, 2], hi, tC)
                mul_b(eng, t_[:, 3], lo, tD)
                eng.tensor_add(qro[:, :, 64:128], t_[:, 2], t_[:, 3])
                psTq = ps_t.tile([128, 4, 128], BF16, tag='pst', name='psTq')
                for hq in range(4):
                    nc.tensor.transpose(psTq[:, hq, :], qro[:, hq, :], iden_s[:])
                nc.scalar.copy(QT[qg * 2][:, :, bass.ts(bi, 128)], psTq[:, 0:2, :])
                nc.scalar.copy(QT[qg * 2 + 1][:, :, bass.ts(bi, 128)], psTq[:, 2:4, :])

        # wo loads (reuse wq slots; runs during attention)
        wo_s = [p_wq.tile([128, 4, 1024], BF16, tag='wq', name='wo') for _ in range(4)]
        for g in range(4):
            nc.gpsimd.dma_start(wo_s[g][:], owT[bass.ts(g, 512), :].rearrange('(n p) c -> p n c', p=128))

        # ---- attention: j-outer, wide score tiles ----
        ctm = [p_wk.tile([128, 2, 1024], BF16, tag='wk', name='ctm') for _ in range(4)]
        # psC slot layout: tag -> (first bi, n slots)
        CGRP = [(0, 3), (3, 3), (6, 2)]

        def cslot(pc, bi):
            g = 0 if bi < 3 else (1 if bi < 6 else 2)
            s = bi - CGRP[g][0]
            return pc[g][:, s * 132:s * 132 + 129]

        ctfs = []
        for h in range(8):
            kvh = h // 2
            qt_ap = QT[h // 2][:, h % 2, :]
            pc = [ps_c.tile([128, CGRP[g][1] * 132], F32, tag=f'c{g}', name=f'pc{g}')
                  for g in range(3)]
            exs = {}
            psTc = None
            for it in range(17):
                # QK for j=it (chunks), exp, mask
                if it < 16:
                    j = it
                    b0 = j // 2
                    if b0 < 4:
                        chunks = [(b0 * 128, 512), (512, 1024)]
                    else:
                        chunks = [(b0 * 128, 1024)]
                    rj = (j % 2) ^ (1 if j >= 8 else 0)
                    kt_ap = KT[:, kvh, rj, bass.ts(j // 2, 128)]
                    cur = []
                    for (c0, c1) in chunks:
                        w = c1 - c0
                        psS = ps_m.tile([128, 512], F32, tag='ps', name='psS')
                        nc.tensor.matmul(psS[:, 0:w], kt_ap, qt_ap[:, c0:c1],
                                         start=True, stop=True)
                        ex = p_ex.tile([128, 512], BF16, tag='ex', name='ex')
                        nc.scalar.activation(ex[:, 0:w], psS[:, 0:w], AF.Exp)
                        cur.append((c0, c1, ex))
                    # diagonal/overhang mask: block bi=b0, t = j parity
                    nc.vector.tensor_mul(cur[0][2][:, 0:128], cur[0][2][:, 0:128],
                                         dm_s[:, b0, j % 2, :])
                    exs[j] = cur
                # PV for j=it-1 (+finalizations)
                if it > 0:
                    j = it - 1
                    b0 = j // 2
                    rj = (j % 2) ^ (1 if j >= 8 else 0)
                    va_ap = VA[:, kvh, rj * 8 + j // 2, 0:129]
                    for bi in range(b0, 8):
                        (c0, c1, ex) = exs[j][0] if bi * 128 < exs[j][0][1] else exs[j][1]
                        exsub = ex[:, bi * 128 - c0:bi * 128 - c0 + 128]
                        # start=True clears has_written for the WHOLE bank: issue it
                        # only on the bank's first matmul; other slots first-write
                        # via the overwrite-where-unwritten path.
                        nc.tensor.matmul(cslot(pc, bi), exsub, va_ap,
                                         start=(j == 0 and bi in (0, 3, 6)),
                                         stop=(j == BOUNDS[bi]),
                                         skip_group_check=True)
                    del exs[j]
                    # finalize bi whose last block was j
                    if it % 2 == 0:
                        bi = (it - 2) // 2
                        sl = cslot(pc, bi)
                        rd = p_s.tile([128, 1], F32, tag='rd', name='rd')
                        nc.vector.reciprocal(rd[:], sl[:, 128:129])
                        cn = p_w.tile([128, 128], BF16, tag='cn', name='cn')
                        nc.vector.tensor_scalar_mul(cn[:], sl[:, 0:128], rd[:])
                        if psTc is None:
                            psTc = ps_t.tile([128, 4, 128], BF16, tag='pst', name='psTc')
                        nc.tensor.transpose(psTc[:, bi % 2, :], cn[:], iden_s[:])
                        if bi % 2 == 1:
                            nc.vector.tensor_scalar_add(
                                ctm[h // 2][:, h % 2, bass.ts(bi // 2, 256)],
                                psTc[:, 0:2, :], 0.0)
                            psTc = None
            # ctx pair-AllGather in 2 chunks (after h3/h7); 512-row shapes -> Mesh
            if h % 4 == 3:
                p = h // 4
                cci = p_d.tile([512, 1024], BF16, tag=f'cci{p}', name=f'cci{p}')
                cco = p_d.tile([1024, 1024], BF16, tag=f'cco{p}', name=f'cco{p}')
                for i in range(2):
                    nc.sync.dma_start(
                        cci[bass.ts(i, 256), :].rearrange('(c p) s -> p c s', c=2),
                        ctm[2 * p + i][:])
                nc.gpsimd.collective_compute(
                    'AllGather', mybir.AluOpType.bypass,
                    replica_groups=[[0, 1], [2, 3], [4, 5], [6, 7]],
                    ins=[cci.opt()], outs=[cco.opt()])
                pool_f, tag_f = (p_h, 'ht') if p == 0 else (p_wv, 'wv')
                cf = [pool_f.tile([128, 2, 1024], BF16, tag=tag_f, name=f'ctf{p}')
                      for _ in range(4)]
                for i in range(4):
                    nc.sync.dma_start(
                        cf[i][:],
                        cco[bass.ts(i, 256), :].rearrange('(c p) s -> p c s', c=2))
                ctfs.append(cf)

        # ---- o_proj: 2 passes (one per ctx chunk), accumulate in SBUF ----
        obuf = p_kv.tile([128, 8, 1024], BF16, tag='obuf', name='obuf')
        for p in range(2):
            cf = ctfs[p]
            for bi in range(8):
                for nt in range(2):
                    psO = ps_m.tile([128, 512], F32, tag='ps', name='psO')
                    for c in range(8):
                        nc.tensor.matmul(psO[:], cf[c // 2][:, c % 2, bass.ts(bi, 128)],
                                         wo_s[2 * p + c // 4][:, c % 4, bass.ts(nt, 512)],
                                         start=(c == 0), stop=(c == 7))
                    dst = obuf[:, bi, bass.ts(nt, 512)]
                    if p == 0:
                        nc.vector.tensor_scalar_add(dst, psO[:], 0.0)
                    else:
                        ob = p_ob.tile([128, 512], F32, tag='ob', name='ob')
                        nc.vector.tensor_add(ob[:], psO[:], dst)
                        nc.sync.dma_start(out_e[bass.ts(bi, 128), bass.ts(nt, 512)], ob[:])

    split_multi_waits(nc)
    return nc


def mul_b(eng, out, a, b):
    """tensor_tensor multiply with free-dim broadcast of b over dim 1."""
    a2, b2 = bass.broadcast_tensor_aps(a, b)
    eng.tensor_mul(out, a2, b2)


# ---------------------------------------------------------------------------
_NC_CACHE = None
_LAST_IN_MAPS = None


def _get_nc():
    global _NC_CACHE
    if _NC_CACHE is None:
        _NC_CACHE = build_kernel()
    return _NC_CACHE


def kernel(hidden_states, cos, sin, q_w, k_w, v_w, o_w, q_norm_w, k_norm_w):
    from concourse.bass_utils import run_bass_kernel_spmd

    hidden_states = np.asarray(hidden_states, np.float32)
    cos = np.asarray(cos, np.float32)
    sin = np.asarray(sin, np.float32)
    q_w = np.asarray(q_w, np.float32)
    k_w = np.asarray(k_w, np.float32)
    v_w = np.asarray(v_w, np.float32)
    o_w = np.asarray(o_w, np.float32)
    q_norm_w = np.asarray(q_norm_w, np.float32)
    k_norm_w = np.asarray(k_norm_w, np.float32)

    tri_np = np.triu(np.ones((128, 128), np.float32))  # [sj,si]: valid sj<=si
    iden_np = np.eye(128, dtype=np.float32)
    operm = np.concatenate([np.arange(h * 128, (h + 1) * 128) for h in OHEAD_ORDER])

    def rope_tabs(c, s_, w):
        # tables [rows, 4, 64]: A=c_lo*w_lo, B=s_lo*w_hi, C=c_lo*w_hi, D=s_lo*w_lo
        cl, sl = c[:, 0:64], s_[:, 0:64]
        wl, wh = w[0:64], w[64:128]
        return np.stack([cl * wl, sl * wh, cl * wh, sl * wl], axis=1).astype(np.float32)

    in_maps = []
    for c in range(8):
        b, sh, hh = c >> 2, (c >> 1) & 1, c & 1
        blks = MYBLKS[sh]
        rows = np.concatenate([np.arange(g * 128, (g + 1) * 128) for g in blks])
        hT = np.ascontiguousarray(hidden_states[b][rows].T)
        qwT = np.ascontiguousarray(q_w[hh * 1024:(hh + 1) * 1024].T)
        kwT = np.ascontiguousarray(k_w[hh * 512:(hh + 1) * 512].T)
        vwT = np.ascontiguousarray(v_w[hh * 512:(hh + 1) * 512].T)
        owT = np.ascontiguousarray(o_w[hh * 1024:(hh + 1) * 1024].T[operm])
        qtab = rope_tabs(cos[b][rows], sin[b][rows], q_norm_w)
        ktab = rope_tabs(cos[b][rows], sin[b][rows], k_norm_w)
        # diagonal masks dm[bi, t]: t=0 -> sj block BOUNDS[bi]-1, t=1 -> BOUNDS[bi]
        dm = np.zeros((8, 2, 128, 128), np.float32)
        for bi in range(8):
            g, gb = blks[bi], BOUNDS[bi]
            for t, j in enumerate((gb - 1, gb)):
                if j < g:
                    dm[bi, t] = 1.0
                elif j == g:
                    dm[bi, t] = tri_np
                # j > g: stays 0 (block fully masked)
        bf = ml_dtypes.bfloat16
        in_maps.append(dict(
            hT=hT.astype(bf), qwT=qwT.astype(bf), kwT=kwT.astype(bf),
            vwT=vwT.astype(bf), owT=owT.astype(bf),
            qtab=qtab.astype(bf), ktab=ktab.astype(bf),
            iden=iden_np.astype(bf), dm=dm.astype(bf)))

    global _LAST_IN_MAPS
    _LAST_IN_MAPS = in_maps
    nc = _get_nc()
    res = run_bass_kernel_spmd(nc, in_maps, core_ids=list(range(8)))

    out = np.zeros((B, S, HID), np.float32)
    for c in range(8):
        b, sh, hh = c >> 2, (c >> 1) & 1, c & 1
        o = res.results[c]['out']  # [1024, 1024]
        for bi, g in enumerate(MYBLKS[sh]):
            out[b, g * 128:(g + 1) * 128, hh * 1024:(hh + 1) * 1024] = \
                o[bi * 128:(bi + 1) * 128]
    return out


if __name__ == '__main__':
    sys.path.insert(0, '/root/problem')
    import reference
    inputs = {k: np.asarray(v) for k, v in reference.setup_inputs().items()}
    exp = np.asarray(reference.reference(**inputs))
    act = kernel(**inputs)
    err = np.abs(act - exp)
    rel = np.linalg.norm(act - exp) / np.linalg.norm(exp)
    print('Relative error:', rel, 'max abs err:', err.max())


# revision 25
# speedup vs baseline: 1.1950x; 1.0036x over previous
"""Trainium2 Bass kernel for Qwen-style GQA attention block (B=2,S=2048,H=16,KV=8,D=128).

Sharding (8 cores): batch(2) x si-stripes(2) x head-half(2).
  core c: b=c>>2, sh=(c>>1)&1, hh=c&1
  - each core projects Q/K/V for ITS stripe rows only (1024 tokens); K/V results
    (roped, transposed, normalized) are exchanged between the two stripe cores
    via a small AllGather so both see full-S K/V.
  - attention j-outer with wide score tiles (stationary K-block reuse, wide exp).
  - pair AllGather of ctx^T split in two head-groups, column-split o_proj in two
    passes so the second collective hides under the first o_proj pass.
All matmuls bf16 with fp32 PSUM accumulation. Softmax without max-subtraction
(scores are O(1) after QK RMSNorm); denominator via an appended ones-column on V.
"""
import sys

sys.path.insert(0, '/opt/trn_rl_repo')

import numpy as np
import ml_dtypes

import concourse.bass as bass
import concourse.tile as tile
from concourse import mybir
from concourse.vector_clock import ScopedClock, VectorClock

B, S, HID = 2, 2048, 2048
H, KV, D = 16, 8, 128
EPS = 1e-6
SCALE = D ** -0.5
NBLK = S // 128  # 16
# causally balanced si-block stripes: sum(i+1) = 68 for both
MYBLKS = [[0, 2, 4, 6, 9, 11, 13, 15], [1, 3, 5, 7, 8, 10, 12, 14]]
BOUNDS = [max(MYBLKS[0][bi], MYBLKS[1][bi]) for bi in range(8)]  # [1,3,..,15]
# o_proj ctx row order after the two pair-AllGathers (global head ids)
OHEAD_ORDER = [0, 1, 2, 3, 8, 9, 10, 11, 4, 5, 6, 7, 12, 13, 14, 15]

F32 = mybir.dt.float32
BF16 = mybir.dt.bfloat16
AF = mybir.ActivationFunctionType
MUL = mybir.AluOpType.mult
ADD = mybir.AluOpType.add


# ---------------------------------------------------------------------------
# Workarounds: this walrus supports only ONE sync-wait per instruction.
def _patched_drain_and_barrier(self, tick_clock, wait_clock):
    gc = tick_clock.global_clock
    vec = list(gc)
    nz = [i for i, v in enumerate(vec) if v > 0] or [0]
    for i in nz:
        cvec = [vec[j] if j == i else 0 for j in range(len(vec))]
        inst = self.nc.sync.drain()
        wait_clock.add_sem_waits(inst.ins, ScopedClock({None: VectorClock(cvec)}))
    self.nc.all_engine_barrier()
    assert self.sems is not None
    popped = self.nc._tile_sem_poison_stack.pop()
    assert popped is self._sem_poison
    self.nc.clear_and_free_semaphores(list(self.sems.allocated().values()))
    self.nc.all_engine_barrier()


tile.TileContext._drain_and_barrier = _patched_drain_and_barrier


def split_multi_waits(nc):
    for fn in nc.m.functions:
        for blk in fn.blocks:
            insts = list(blk.instructions)
            out = []
            changed = False
            for inst in insts:
                si = inst.sync_info
                if si is not None and len(si.on_wait) > 1:
                    waits = list(si.on_wait)
                    for k, w in enumerate(waits[:-1]):
                        out.append(mybir.InstNoOp(
                            name=f"{inst.name}.w{k}", engine=inst.engine,
                            sync_info=mybir.SyncInfo(on_wait=[w], on_update=[]),
                            text_hint="waitsplit"))
                    si.on_wait = [waits[-1]]
                    changed = True
                out.append(inst)
            if changed:
                blk.instructions[:] = out


# ---------------------------------------------------------------------------
def build_kernel():
    nc = bass.Bass(trn_type='TRN2')
    # hidden^T for THIS core's stripe rows (local bi-block order)
    hT = nc.dram_tensor('hT', [HID, 1024], BF16, kind='ExternalInput')
    qwT = nc.dram_tensor('qwT', [HID, 1024], BF16, kind='ExternalInput')
    kwT = nc.dram_tensor('kwT', [HID, 512], BF16, kind='ExternalInput')
    vwT = nc.dram_tensor('vwT', [HID, 512], BF16, kind='ExternalInput')
    owT = nc.dram_tensor('owT', [2048, 1024], BF16, kind='ExternalInput')
    # host-fused rope tables (cos/sin x norm-weight halves), [1024, 4, 64]
    qtab = nc.dram_tensor('qtab', [1024, 4, 64], BF16, kind='ExternalInput')
    ktab = nc.dram_tensor('ktab', [1024, 4, 64], BF16, kind='ExternalInput')
    iden = nc.dram_tensor('iden', [128, 128], BF16, kind='ExternalInput')
    # per-core diagonal masks dm[bi, t] for j in {BOUNDS[bi]-1, BOUNDS[bi]}
    dm = nc.dram_tensor('dm', [8, 2, 128, 128], BF16, kind='ExternalInput')
    out_e = nc.dram_tensor('out', [1024, 1024], F32, kind='ExternalOutput')

    from contextlib import ExitStack
    with ExitStack() as ctx:
        tc = ctx.enter_context(tile.TileContext(nc))
        pool = lambda name, bufs, **kw: ctx.enter_context(
            tc.tile_pool(name=name, bufs=bufs, **kw))
        p_c = pool('const', 1)
        p_wv = pool('wv', 4)      # wv tiles, later QT tiles
        p_wk = pool('wk', 4)      # wk tiles, later ctm tiles
        p_wq = pool('wq', 4)      # wq tiles, later wo tiles
        p_h = pool('ht', 4)       # hT tiles, later ctf tiles
        p_kv = pool('kvstage', 1)  # KTh/VAh staging + KT_all/VA_all
        p_w = pool('work', 2)     # rope scratch etc
        p_s = pool('small', 4)
        p_ex = pool('expb', 4)
        p_ob = pool('outb', 2)
        ps_m = pool('psM', 3, space='PSUM')   # proj psums + scores + o_proj
        ps_c = pool('psC', 1, space='PSUM')   # ctx accumulators (3 tags)
        ps_t = pool('psT', 2, space='PSUM')   # transposes
        p_d = pool('dram', 1, space='DRAM')

        # ---- DMA prologue (gpsimd queue): wv/hT first so V proj starts early
        wv_s = [p_wv.tile([128, 4, 512], BF16, tag='wv', name='wv') for _ in range(4)]
        wk_s = [p_wk.tile([128, 4, 512], BF16, tag='wk', name='wk') for _ in range(4)]
        wq_s = [p_wq.tile([128, 4, 1024], BF16, tag='wq', name='wq') for _ in range(4)]
        ht_t = [p_h.tile([128, 4, 1024], BF16, tag='ht', name='ht') for _ in range(4)]
        for g in range(4):
            r = bass.ts(g, 512)
            nc.gpsimd.dma_start(wv_s[g][:], vwT[r, :].rearrange('(n p) c -> p n c', p=128))
            nc.gpsimd.dma_start(ht_t[g][:], hT[r, :].rearrange('(n p) c -> p n c', p=128))
        for g in range(4):
            nc.gpsimd.dma_start(wk_s[g][:], kwT[bass.ts(g, 512), :].rearrange('(n p) c -> p n c', p=128))
        iden_s = p_c.tile([128, 128], BF16)
        nc.gpsimd.dma_start(iden_s[:], iden[:])
        ktab_s = p_c.tile([128, 8, 4, 64], BF16)
        nc.gpsimd.dma_start(ktab_s[:], ktab.rearrange('(n p) t d -> p n t d', p=128))
        qtab_s = p_c.tile([128, 8, 4, 64], BF16)
        nc.gpsimd.dma_start(qtab_s[:], qtab.rearrange('(n p) t d -> p n t d', p=128))
        for g in range(4):
            nc.gpsimd.dma_start(wq_s[g][:], qwT[bass.ts(g, 512), :].rearrange('(n p) c -> p n c', p=128))
        dm_s = p_c.tile([128, 8, 2, 128], BF16)
        nc.gpsimd.dma_start(dm_s[:], dm.rearrange('n t p d -> p n t d'))

        # persistent K/V stores (full S, post-exchange) + local staging
        KTh = p_kv.tile([128, 4, 1024], BF16, tag='kth', name='KTh')
        VAh = p_kv.tile([128, 4, 8, 132], BF16, tag='vah', name='VAh')
        KT = p_kv.tile([128, 4, 2, 1024], BF16, tag='kt', name='KT')
        VA = p_kv.tile([128, 4, 16, 132], BF16, tag='va', name='VA')
        nc.gpsimd.memset(VAh[:, :, :, 128:132], 1.0)

        # ---- V projection: ch-major waves so PE starts on the first DMA chunk
        wave_tags = ['ps', 'ps', 'ps', 'c0', 'c1', 'c2']
        for wave, sbs in ((0, range(0, 6)), (1, range(6, 8))):
            psVs = {}
            for idx, sb in enumerate(sbs):
                tag = wave_tags[idx] if wave == 0 else 'ps'
                pool_ = ps_m if tag == 'ps' else ps_c
                psVs[sb] = pool_.tile([128, 512], F32, tag=tag, name=f'psV{sb}')
            for g in range(4):
                for i in range(4):
                    ch = g * 4 + i
                    for sb in sbs:
                        nc.tensor.matmul(psVs[sb][:], ht_t[g][:, i, bass.ts(sb, 128)],
                                         wv_s[g][:, i, :],
                                         start=(ch == 0), stop=(ch == 15))
            for sb in sbs:
                nc.scalar.copy(VAh[:, :, sb, 0:128],
                               psVs[sb][:].rearrange('p (k d) -> p k d', k=4))

        # ---- V exchange: launch immediately so even a slow Ring hides ----
        ccV_in = p_d.tile([512, 1056], BF16, tag='cvi', name='ccV_in')
        ccV_out = p_d.tile([1024, 1056], BF16, tag='cvo', name='ccV_out')
        nc.sync.dma_start(ccV_in[:].rearrange('(p c) s -> p c s', p=128),
                          VAh[:].rearrange('p k s w -> p k (s w)'))
        nc.gpsimd.collective_compute(
            'AllGather', mybir.AluOpType.bypass,
            replica_groups=[[0, 2], [1, 3], [4, 6], [5, 7]],
            ins=[ccV_in.opt()], outs=[ccV_out.opt()])
        for r in range(2):
            nc.sync.dma_start(
                VA[:, :, r * 8:(r + 1) * 8, :].rearrange('p k s w -> p k (s w)'),
                ccV_out[bass.ts(r, 512), :].rearrange('(p c) s -> p c s', p=128))

        # ---- K projection + RMSNorm(*SCALE) + rope + transpose ----
        for sb in range(8):
            ktag = 'ps' if sb % 2 == 0 else f'c{(sb // 2) % 3}'
            kpool = ps_m if ktag == 'ps' else ps_c
            psK = kpool.tile([128, 512], F32, tag=ktag, name='psK')
            for ch in range(16):
                nc.tensor.matmul(psK[:], ht_t[ch // 4][:, ch % 4, bass.ts(sb, 128)],
                                 wk_s[ch // 4][:, ch % 4, :],
                                 start=(ch == 0), stop=(ch == 15))
            kraw = p_w.tile([128, 4, 128], BF16, tag='raw', name='kraw')
            nc.scalar.copy(kraw[:], psK[:].rearrange('p (k d) -> p k d', k=4))
            sqd = p_w.tile([128, 4, 128], BF16, tag='sqd', name='sqd')
            nc.vector.tensor_mul(sqd[:], kraw[:], kraw[:])
            ms = p_s.tile([128, 4], F32, tag='ms', name='ms')
            nc.vector.tensor_reduce(ms[:], sqd[:], mybir.AxisListType.X, ADD)
            nc.vector.tensor_scalar_add(ms[:], ms[:], float(EPS * D))
            std = p_s.tile([128, 4], F32, tag='std', name='std')
            nc.scalar.activation(std[:], ms[:], AF.Sqrt, scale=1.0 / D, bias=0.0)
            rstd = p_s.tile([128, 4], F32, tag='rstd', name='rstd')
            nc.vector.reciprocal(rstd[:], std[:])
            rstdS = p_s.tile([128, 4], F32, tag='rstds', name='rstdS')
            nc.vector.tensor_scalar_mul(rstdS[:], rstd[:], SCALE)
            kcs = p_w.tile([128, 4, 128], BF16, tag='kcs', name='kcs')
            for kvh in range(4):
                nc.vector.tensor_scalar_mul(kcs[:, kvh, :], kraw[:, kvh, :],
                                            rstdS[:, kvh:kvh + 1])
            # rope on gpsimd (keeps DVE free); tables already fold k_norm_w
            lo, hi = kcs[:, :, 0:64], kcs[:, :, 64:128]
            tA = ktab_s[:, sb, :, :][:, 0:1, :]
            tB = ktab_s[:, sb, :, :][:, 1:2, :]
            tC = ktab_s[:, sb, :, :][:, 2:3, :]
            tD = ktab_s[:, sb, :, :][:, 3:4, :]
            t_ = p_w.tile([128, 4, 4, 64], BF16, tag='t4', name='t4')
            kro = p_w.tile([128, 4, 128], BF16, tag='kro', name='kro')
            mul_b(nc.vector, t_[:, 0], lo, tA)
            mul_b(nc.vector, t_[:, 1], hi, tB)
            nc.vector.tensor_sub(kro[:, :, 0:64], t_[:, 0], t_[:, 1])
            mul_b(nc.vector, t_[:, 2], hi, tC)
            mul_b(nc.vector, t_[:, 3], lo, tD)
            nc.vector.tensor_add(kro[:, :, 64:128], t_[:, 2], t_[:, 3])
            psTk = ps_t.tile([128, 4, 128], BF16, tag='pst', name='psTk')
            for kvh in range(4):
                nc.tensor.transpose(psTk[:, kvh, :], kro[:, kvh, :], iden_s[:])
            nc.scalar.copy(KTh[:, :, bass.ts(sb, 128)], psTk[:])

        # K exchange is emitted inside the Q loop (after bi==2) so the
        # AllGather trigger's input-DMA wait cannot stall early gpsimd work.
        def emit_k_exchange():
            ccK_in = p_d.tile([512, 1024], BF16, tag='cki', name='ccK_in')
            ccK_out = p_d.tile([1024, 1024], BF16, tag='cko', name='ccK_out')
            nc.sync.dma_start(ccK_in[:].rearrange('(p c) s -> p c s', p=128), KTh[:])
            nc.gpsimd.collective_compute(
                'AllGather', mybir.AluOpType.bypass,
                replica_groups=[[0, 2], [1, 3], [4, 6], [5, 7]],
                ins=[ccK_in.opt()], outs=[ccK_out.opt()])
            for r in range(2):
                nc.sync.dma_start(
                    KT[:, :, r, :],
                    ccK_out[bass.ts(r, 512), :].rearrange('(p c) s -> p c s', p=128))

        # ---- Q projection (PE busy while exchange completes) ----
        QT = [p_wv.tile([128, 2, 1024], BF16, tag='wv', name='QT') for _ in range(4)]
        for bi in range(8):
            if bi == 3:
                emit_k_exchange()
            for qg in range(2):
                qtag = 'ps' if (bi + qg) % 2 == 0 else f'c{bi % 3}'
                qpool = ps_m if qtag == 'ps' else ps_c
                psQ = qpool.tile([128, 512], F32, tag=qtag, name='psQ')
                for ch in range(16):
                    nc.tensor.matmul(psQ[:], ht_t[ch // 4][:, ch % 4, bass.ts(bi, 128)],
                                     wq_s[ch // 4][:, ch % 4, bass.ts(qg, 512)],
                                     start=(ch == 0), stop=(ch == 15))
                qraw = p_w.tile([128, 4, 128], BF16, tag='raw', name='qraw')
                nc.scalar.copy(qraw[:], psQ[:].rearrange('p (k d) -> p k d', k=4))
                sqd = p_w.tile([128, 4, 128], BF16, tag='sqd', name='sqd')
                nc.vector.tensor_mul(sqd[:], qraw[:], qraw[:])
                ms = p_s.tile([128, 4], F32, tag='ms', name='ms')
                nc.vector.tensor_reduce(ms[:], sqd[:], mybir.AxisListType.X, ADD)
                nc.vector.tensor_scalar_add(ms[:], ms[:], float(EPS * D))
                std = p_s.tile([128, 4], F32, tag='std', name='std')
                nc.scalar.activation(std[:], ms[:], AF.Sqrt, scale=1.0 / D, bias=0.0)
                rstd = p_s.tile([128, 4], F32, tag='rstd', name='rstd')
                nc.vector.reciprocal(rstd[:], std[:])
                qcs = p_w.tile([128, 4, 128], BF16, tag='kcs', name='qcs')
                for hq in range(4):
                    nc.vector.tensor_scalar_mul(qcs[:, hq, :], qraw[:, hq, :],
                                                rstd[:, hq:hq + 1])
                lo, hi = qcs[:, :, 0:64], qcs[:, :, 64:128]
                tA = qtab_s[:, bi, :, :][:, 0:1, :]
                tB = qtab_s[:, bi, :, :][:, 1:2, :]
                tC = qtab_s[:, bi, :, :][:, 2:3, :]
                tD = qtab_s[:, bi, :, :][:, 3:4, :]
                t_ = p_w.tile([128, 4, 4, 64], BF16, tag='t4', name='t4')
                qro = p_w.tile([128, 4, 128], BF16, tag='kro', name='qro')
                eng = nc.vector if qg == 0 else nc.gpsimd
                mul_b(eng, t_[:, 0], lo, tA)
                mul_b(eng, t_[:, 1], hi, tB)
                eng.tensor_sub(qro[:, :, 0:64], t_[:, 0], t_[:, 1])
                mul_b(eng, t_[:, 2], hi, tC)
                mul_b(eng, t_[:, 3], lo, tD)
                eng.tensor_add(qro[:, :, 64:128], t_[:, 2], t_[:, 3])
                psTq = ps_t.tile([128, 4, 128], BF16, tag='pst', name='psTq')
                for hq in range(4):
                    nc.tensor.transpose(psTq[:, hq, :], qro[:, hq, :], iden_s[:])
                nc.scalar.copy(QT[qg * 2][:, :, bass.ts(bi, 128)], psTq[:, 0:2, :])
                nc.scalar.copy(QT[qg * 2 + 1][:, :, bass.ts(bi, 128)], psTq[:, 2:4, :])

        # wo loads (reuse wq slots; runs during attention)
        wo_s = [p_wq.tile([128, 4, 1024], BF16, tag='wq', name='wo') for _ in range(4)]
        for g in range(4):
            nc.gpsimd.dma_start(wo_s[g][:], owT[bass.ts(g, 512), :].rearrange('(n p) c -> p n c', p=128))

        # ---- attention: j-outer, wide score tiles ----
        ctm = [p_wk.tile([128, 2, 1024], BF16, tag='wk', name='ctm') for _ in range(4)]
        # psC slot layout: tag -> (first bi, n slots)
        CGRP = [(0, 3), (3, 3), (6, 2)]

        def cslot(pc, bi):
            g = 0 if bi < 3 else (1 if bi < 6 else 2)
            s = bi - CGRP[g][0]
            return pc[g][:, s * 132:s * 132 + 129]

        ctfs = []
        for h in range(8):
            kvh = h // 2
            qt_ap = QT[h // 2][:, h % 2, :]
            pc = [ps_c.tile([128, CGRP[g][1] * 132], F32, tag=f'c{g}', name=f'pc{g}')
                  for g in range(3)]
            exs = {}
            psTc = None
            for it in range(17):
                # QK for j=it (chunks), exp, mask
                if it < 16:
                    j = it
                    b0 = j // 2
                    if b0 < 4:
                        chunks = [(b0 * 128, 512), (512, 1024)]
                    else:
                        chunks = [(b0 * 128, 1024)]
                    rj = (j % 2) ^ (1 if j >= 8 else 0)
                    kt_ap = KT[:, kvh, rj, bass.ts(j // 2, 128)]
                    cur = []
                    for (c0, c1) in chunks:
                        w = c1 - c0
                        psS = ps_m.tile([128, 512], F32, tag='ps', name='psS')
                        nc.tensor.matmul(psS[:, 0:w], kt_ap, qt_ap[:, c0:c1],
                                         start=True, stop=True)
                        ex = p_ex.tile([128, 512], BF16, tag='ex', name='ex')
                        nc.scalar.activation(ex[:, 0:w], psS[:, 0:w], AF.Exp)
                        cur.append((c0, c1, ex))
                    # diagonal/overhang mask: block bi=b0, t = j parity
                    nc.gpsimd.tensor_mul(cur[0][2][:, 0:128], cur[0][2][:, 0:128],
                                         dm_s[:, b0, j % 2, :])
                    exs[j] = cur
                # PV for j=it-1 (+finalizations)
                if it > 0:
                    j = it - 1
                    b0 = j // 2
                    rj = (j % 2) ^ (1 if j >= 8 else 0)
                    va_ap = VA[:, kvh, rj * 8 + j // 2, 0:129]
                    for bi in range(b0, 8):
                        (c0, c1, ex) = exs[j][0] if bi * 128 < exs[j][0][1] else exs[j][1]
                        exsub = ex[:, bi * 128 - c0:bi * 128 - c0 + 128]
                        # start=True clears has_written for the WHOLE bank: issue it
                        # only on the bank's first matmul; other slots first-write
                        # via the overwrite-where-unwritten path.
                        nc.tensor.matmul(cslot(pc, bi), exsub, va_ap,
                                         start=(j == 0 and bi in (0, 3, 6)),
                                         stop=(j == BOUNDS[bi]),
                                         skip_group_check=True)
                    del exs[j]
                    # finalize bi whose last block was j
                    if it % 2 == 0:
                        bi = (it - 2) // 2
                        sl = cslot(pc, bi)
                        rd = p_s.tile([128, 1], F32, tag='rd', name='rd')
                        nc.vector.reciprocal(rd[:], sl[:, 128:129])
                        cn = p_w.tile([128, 128], BF16, tag='cn', name='cn')
                        nc.vector.tensor_scalar_mul(cn[:], sl[:, 0:128], rd[:])
                        if psTc is None:
                            psTc = ps_t.tile([128, 4, 128], BF16, tag='pst', name='psTc')
                        nc.tensor.transpose(psTc[:, bi % 2, :], cn[:], iden_s[:])
                        if bi % 2 == 1:
                            nc.vector.tensor_scalar_add(
                                ctm[h // 2][:, h % 2, bass.ts(bi // 2, 256)],
                                psTc[:, 0:2, :], 0.0)
                            psTc = None
            # ctx pair-AllGather in 2 chunks (after h3/h7); 512-row shapes -> Mesh
            if h % 4 == 3:
                p = h // 4
                cci = p_d.tile([512, 1024], BF16, tag=f'cci{p}', name=f'cci{p}')
                cco = p_d.tile([1024, 1024], BF16, tag=f'cco{p}', name=f'cco{p}')
                for i in range(2):
                    nc.sync.dma_start(
                        cci[bass.ts(i, 256), :].rearrange('(c p) s -> p c s', c=2),
                        ctm[2 * p + i][:])
                nc.gpsimd.collective_compute(
                    'AllGather', mybir.AluOpType.bypass,
                    replica_groups=[[0, 1], [2, 3], [4, 5], [6, 7]],
                    ins=[cci.opt()], outs=[cco.opt()])
                pool_f, tag_f = (p_h, 'ht') if p == 0 else (p_wv, 'wv')
                cf = [pool_f.tile([128, 2, 1024], BF16, tag=tag_f, name=f'ctf{p}')
                      for _ in range(4)]
                for i in range(4):
                    nc.sync.dma_start(
                        cf[i][:],
                        cco[bass.ts(i, 256), :].rearrange('(c p) s -> p c s', c=2))
                ctfs.append(cf)

        # ---- o_proj: 2 passes (one per ctx chunk), accumulate in SBUF ----
        obuf = p_kv.tile([128, 8, 1024], BF16, tag='obuf', name='obuf')
        for p in range(2):
            cf = ctfs[p]
            for bi in range(8):
                for nt in range(2):
                    psO = ps_m.tile([128, 512], F32, tag='ps', name='psO')
                    for c in range(8):
                        nc.tensor.matmul(psO[:], cf[c // 2][:, c % 2, bass.ts(bi, 128)],
                                         wo_s[2 * p + c // 4][:, c % 4, bass.ts(nt, 512)],
                                         start=(c == 0), stop=(c == 7))
                    dst = obuf[:, bi, bass.ts(nt, 512)]
                    if p == 0:
                        nc.vector.tensor_scalar_add(dst, psO[:], 0.0)
                    else:
                        ob = p_ob.tile([128, 512], F32, tag='ob', name='ob')
                        nc.vector.tensor_add(ob[:], psO[:], dst)
                        nc.sync.dma_start(out_e[bass.ts(bi, 128), bass.ts(nt, 512)], ob[:])

    split_multi_waits(nc)
    return nc


def mul_b(eng, out, a, b):
    """tensor_tensor multiply with free-dim broadcast of b over dim 1."""
    a2, b2 = bass.broadcast_tensor_aps(a, b)
    eng.tensor_mul(out, a2, b2)


# ---------------------------------------------------------------------------
_NC_CACHE = None
_LAST_IN_MAPS = None


def _get_nc():
    global _NC_CACHE
    if _NC_CACHE is None:
        _NC_CACHE = build_kernel()
    return _NC_CACHE


def kernel(hidden_states, cos, sin, q_w, k_w, v_w, o_w, q_norm_w, k_norm_w):
    from concourse.bass_utils import run_bass_kernel_spmd

    hidden_states = np.asarray(hidden_states, np.float32)
    cos = np.asarray(cos, np.float32)
    sin = np.asarray(sin, np.float32)
    q_w = np.asarray(q_w, np.float32)
    k_w = np.asarray(k_w, np.float32)
    v_w = np.asarray(v_w, np.float32)
    o_w = np.asarray(o_w, np.float32)
    q_norm_w = np.asarray(q_norm_w, np.float32)
    k_norm_w = np.asarray(k_norm_w, np.float32)

    tri_np = np.triu(np.ones((128, 128), np.float32))  # [sj,si]: valid sj<=si
    iden_np = np.eye(128, dtype=np.float32)
    operm = np.concatenate([np.arange(h * 128, (h + 1) * 128) for h in OHEAD_ORDER])

    def rope_tabs(c, s_, w):
        # tables [rows, 4, 64]: A=c_lo*w_lo, B=s_lo*w_hi, C=c_lo*w_hi, D=s_lo*w_lo
        cl, sl = c[:, 0:64], s_[:, 0:64]
        wl, wh = w[0:64], w[64:128]
        return np.stack([cl * wl, sl * wh, cl * wh, sl * wl], axis=1).astype(np.float32)

    in_maps = []
    for c in range(8):
        b, sh, hh = c >> 2, (c >> 1) & 1, c & 1
        blks = MYBLKS[sh]
        rows = np.concatenate([np.arange(g * 128, (g + 1) * 128) for g in blks])
        hT = np.ascontiguousarray(hidden_states[b][rows].T)
        qwT = np.ascontiguousarray(q_w[hh * 1024:(hh + 1) * 1024].T)
        kwT = np.ascontiguousarray(k_w[hh * 512:(hh + 1) * 512].T)
        vwT = np.ascontiguousarray(v_w[hh * 512:(hh + 1) * 512].T)
        owT = np.ascontiguousarray(o_w[hh * 1024:(hh + 1) * 1024].T[operm])
        qtab = rope_tabs(cos[b][rows], sin[b][rows], q_norm_w)
        ktab = rope_tabs(cos[b][rows], sin[b][rows], k_norm_w)
        # diagonal masks dm[bi, t]: t=0 -> sj block BOUNDS[bi]-1, t=1 -> BOUNDS[bi]
        dm = np.zeros((8, 2, 128, 128), np.float32)
        for bi in range(8):
            g, gb = blks[bi], BOUNDS[bi]
            for t, j in enumerate((gb - 1, gb)):
                if j < g:
                    dm[bi, t] = 1.0
                elif j == g:
                    dm[bi, t] = tri_np
                # j > g: stays 0 (block fully masked)
        bf = ml_dtypes.bfloat16
        in_maps.append(dict(
            hT=hT.astype(bf), qwT=qwT.astype(bf), kwT=kwT.astype(bf),
            vwT=vwT.astype(bf), owT=owT.astype(bf),
            qtab=qtab.astype(bf), ktab=ktab.astype(bf),
            iden=iden_np.astype(bf), dm=dm.astype(bf)))

    global _LAST_IN_MAPS
    _LAST_IN_MAPS = in_maps
    nc = _get_nc()
    res = run_bass_kernel_spmd(nc, in_maps, core_ids=list(range(8)))

    out = np.zeros((B, S, HID), np.float32)
    for c in range(8):
        b, sh, hh = c >> 2, (c >> 1) & 1, c & 1
        o = res.results[c]['out']  # [1024, 1024]
        for bi, g in enumerate(MYBLKS[sh]):
            out[b, g * 128:(g + 1) * 128, hh * 1024:(hh + 1) * 1024] = \
                o[bi * 128:(bi + 1) * 128]
    return out


if __name__ == '__main__':
    sys.path.insert(0, '/root/problem')
    import reference
    inputs = {k: np.asarray(v) for k, v in reference.setup_inputs().items()}
    exp = np.asarray(reference.reference(**inputs))
    act = kernel(**inputs)
    err = np.abs(act - exp)
    rel = np.linalg.norm(act - exp) / np.linalg.norm(exp)
    print('Relative error:', rel, 'max abs err:', err.max())
